# revision 33
# baseline (speedup 1.0000x reference)
"""GroupShuffleAttention Trainium2 kernel.

Per-core = one batch (B=8 over 8 cores). Layout tricks:
- grouped 1x1 conv as one dense matmul with a block-structured, 32-row-spaced
  weight ("T2" layout: quad q holds groups 4q+j at partition rows 32j..32j+15).
- per-group NxN scores via row-tiled (tile_position) f32r matmuls; the score
  matrix is symmetric so the same stored exp(scores) tiles serve as both
  softmax rows and the attn@V contraction operand -- no NxN transposes.
- softmax denominator Z comes free as a 17th "ones" column in the attn@V
  stationary operand (col-tiled matmuls place each group at psum rows 32j).
- elu(t) = relu(t) + exp(min(t,0)) - 1; the -1 contributes exactly -1 to the
  attention output (att columns sum to 1) and then cancels in GroupNorm
  (shift-invariant), so it is dropped entirely.
- GroupNorm stats and per-partition broadcast via tiny indicator matmuls.
"""

import os
import sys

sys.path.insert(0, "/opt/trn_rl_repo")

import numpy as np

import concourse.bass as bass
import concourse.mybir as mybir
import concourse.tile as tile
from concourse import bacc
from concourse.bass_utils import run_bass_kernel_spmd

f32 = mybir.dt.float32
f32r = mybir.dt.float32r
bf16 = mybir.dt.bfloat16
AF = mybir.ActivationFunctionType
ALU = mybir.AluOpType

B, C, N, G = 8, 128, 1024, 8
CG = 16
EPS = 1e-5

_CACHE = {}


def _build_nc(reps=1, ablate=0):
    nc = bacc.Bacc("TRN2", target_bir_lowering=False)

    pts_d = nc.dram_tensor("pts", [C, N], f32r, kind="ExternalInput")
    pts2_d = nc.dram_tensor("pts2", [C, N], f32, kind="ExternalInput")
    w2t_d = nc.dram_tensor("w2t", [2, 128, 128], f32r, kind="ExternalInput")
    bias2_d = nc.dram_tensor("bias2", [2, 128], f32, kind="ExternalInput")
    ident_d = nc.dram_tensor("ident", [128, 128], f32r, kind="ExternalInput")
    identb_d = nc.dram_tensor("identb", [128, 128], bf16, kind="ExternalInput")
    indq_d = nc.dram_tensor("indq", [4, 128], f32r, kind="ExternalInput")
    indz_d = nc.dram_tensor("indz", [128, 4], f32r, kind="ExternalInput")
    indic_d = nc.dram_tensor("indic", [128, 8], f32r, kind="ExternalInput")
    indt_d = nc.dram_tensor("indt", [8, 128], f32r, kind="ExternalInput")
    gb_d = nc.dram_tensor("gb", [2, 128, 2], f32, kind="ExternalInput")
    out_d = nc.dram_tensor("out", [C, N], f32, kind="ExternalOutput")

    bf = {
        "t2p": int(os.environ.get("GSA_B_T2P", "2")),
        "vtp": int(os.environ.get("GSA_B_VTP", "2")),
        "elut": int(os.environ.get("GSA_B_ELUT", "2")),
        "ep": int(os.environ.get("GSA_B_EP", "2")),
        "misc": int(os.environ.get("GSA_B_MISC", "2")),
        "scps": int(os.environ.get("GSA_B_SCPS", "2")),
        "avps": int(os.environ.get("GSA_B_AVPS", "2")),
        "unroll": int(os.environ.get("GSA_UNROLL", "2")),
    }
    with tile.TileContext(nc) as tc:
        with tc.tile_pool(name="consts", bufs=1) as cp, \
             tc.tile_pool(name="t2p", bufs=bf["t2p"]) as t2p, \
             tc.tile_pool(name="vtp", bufs=bf["vtp"]) as vtp, \
             tc.tile_pool(name="elut", bufs=bf["elut"]) as elut, \
             tc.tile_pool(name="ep", bufs=bf["ep"]) as ep, \
             tc.tile_pool(name="misc", bufs=bf["misc"]) as misc, \
             tc.tile_pool(name="scps", bufs=bf["scps"], space="PSUM") as scps, \
             tc.tile_pool(name="avps", bufs=bf["avps"], space="PSUM") as avps, \
             tc.tile_pool(name="trbc", bufs=2, space="PSUM") as trbc:

            # ---- load constants / inputs ----
            pts_sb = cp.tile([128, N], f32r)
            nc.sync.dma_start(out=pts_sb, in_=pts_d[:])
            w2t_sb = cp.tile([128, 2, 128], f32r)
            nc.sync.dma_start(
                out=w2t_sb,
                in_=bass.AP(tensor=w2t_d, offset=0,
                            ap=[[128, 128], [128 * 128, 2], [1, 128]]))
            bias2_sb = cp.tile([128, 2], f32)
            nc.sync.dma_start(
                out=bias2_sb,
                in_=bass.AP(tensor=bias2_d, offset=0, ap=[[1, 128], [128, 2]]))
            ident_sb = cp.tile([128, 128], f32r)
            nc.sync.dma_start(out=ident_sb, in_=ident_d[:])
            identb_sb = cp.tile([128, 128], bf16)
            nc.sync.dma_start(out=identb_sb, in_=identb_d[:])
            indq_sb = cp.tile([4, 128], f32r)
            nc.sync.dma_start(out=indq_sb, in_=indq_d[:])
            indz_sb = cp.tile([128, 4], f32r)
            nc.sync.dma_start(out=indz_sb, in_=indz_d[:])
            indic_sb = cp.tile([128, 8], f32r)
            nc.sync.dma_start(out=indic_sb, in_=indic_d[:])
            indt_sb = cp.tile([8, 128], f32r)
            nc.sync.dma_start(out=indt_sb, in_=indt_d[:])
            gb_sb = cp.tile([128, 2, 2], f32)
            nc.sync.dma_start(
                out=gb_sb,
                in_=bass.AP(tensor=gb_d, offset=0,
                            ap=[[2, 128], [256, 2], [1, 2]]))
            # fast-rsqrt magic constant (int32) for the GroupNorm rstd
            i32 = mybir.dt.int32
            magic_sb = cp.tile([8, 1], i32)
            nc.vector.memset(magic_sb, 0x5F3759DF)
            # shuffled-channel residual input: partition 32j+c of quad q holds
            # points channel c*8+(4q+j)
            ptsq_sb = cp.tile([128, 2, N], f32)
            nc.vector.memset(ptsq_sb, 0.0)
            for q in range(2):
                for j in range(4):
                    nc.sync.dma_start(
                        out=ptsq_sb[32 * j:32 * j + 16, q, :],
                        in_=bass.AP(tensor=pts2_d, offset=(4 * q + j) * N,
                                    ap=[[8 * N, 16], [1, N]]))

            def rep_body():
                # ---- conv (grouped 1x1) into T2 layout + bias ----
                t2_sb = t2p.tile([128, 2, N], f32r, tag="t2")
                for q in range(2):
                    for h2 in range(2):
                        cps = scps.tile([128, 512], f32, tag="sc")
                        nc.tensor.matmul(
                            out=cps[:],
                            lhsT=w2t_sb[:, q, :],
                            rhs=pts_sb[:, h2 * 512:(h2 + 1) * 512],
                            start=True, stop=True)
                        nc.vector.tensor_scalar(
                            out=t2_sb[:, q, h2 * 512:(h2 + 1) * 512], in0=cps[:],
                            scalar1=bias2_sb[:, q:q + 1],
                            scalar2=None, op0=ALU.add)

                # ---- transposes (for V^T) + elu ----
                # vT layout per quad: [128 n-sub, k-tile, strip j, 32] where cols
                # 0..15 = elu(t), col 16 = ones (for Z row)
                # elu reads min/max straight from the transpose PSUM (no staging
                # copy); 4 transposes share one PSUM tile per half.
                vt_sb = vtp.tile([128, 2, 8, 4, 32], bf16, tag="vt")
                for q in range(2):
                    mn = elut.tile([128, N], f32, tag="e1")
                    mx = elut.tile([128, N], f32, tag="e2")
                    for g4 in range(2):
                        trp = trbc.tile([128, 512], f32r, tag="tb")
                        for kk in range(4):
                            k = 4 * g4 + kk
                            nc.tensor.transpose(
                                out=trp[:, kk * 128:(kk + 1) * 128],
                                in_=t2_sb[:, q, k * 128:(k + 1) * 128],
                                identity=ident_sb[:])
                        nc.vector.tensor_scalar_min(
                            mn[:, g4 * 512:(g4 + 1) * 512], trp[:], 0.0)
                        nc.vector.tensor_scalar_max(
                            mx[:, g4 * 512:(g4 + 1) * 512], trp[:], 0.0)
                    ex = elut.tile([128, N], f32, tag="e3")
                    nc.scalar.activation(out=ex[:], in_=mn[:], func=AF.Exp)
                    nc.gpsimd.tensor_add(
                        out=vt_sb[:, q, :, :, :].rearrange("p k j c -> p (k j c)"),
                        in0=mx[:], in1=ex[:])
                nc.gpsimd.memset(vt_sb[:, :, :, :, 16:17], 1.0)

                # ---- scores + exp + attn@V, software-pipelined ----
                # Block p emits scores(p+1) chunks interleaved with attnV(p)
                # matmuls so the PE queue never drains while Act runs the
                # exps for the NEXT pair: without this the in-order PE queue
                # stalls on exp(p) before it can start scores(p+1).
                av_tiles = {}
                et_tiles = {}
                y_sb = t2p.tile([128, 2, N], f32, tag="y")
                sums = misc.tile([128, 4], f32r, tag="sums")

                def emit_scores_chunk(p, i, h):
                    q, lp = p // 2, p % 2
                    et = et_tiles[p]
                    sps = scps.tile([128, N], f32, tag="sc")
                    for rt in range(2):
                        j = 2 * lp + rt
                        nc.tensor.matmul(
                            out=sps[:, rt * 512:(rt + 1) * 512],
                            lhsT=t2_sb[32 * j:32 * j + 32, q,
                                       i * 128:(i + 1) * 128],
                            rhs=t2_sb[32 * j:32 * j + 32, q,
                                      h * 512:(h + 1) * 512],
                            start=True, stop=True,
                            tile_position=(32 * j, 0))
                    nc.scalar.activation(
                        out=et[:, i, h, :, :],
                        in_=sps[:].rearrange("p (r n) -> p r n", r=2),
                        func=AF.Exp, scale=0.25)

                # upper-right superblock (row blocks 0..3 x col blocks 4..7)
                # is the transpose of the computed lower-left; materialize it
                # with PE transposes of the exp'd bf16 tiles + one DVE 2x
                # copy per pair of row blocks, skipping 4 exp chunks per pair.
                def gen_mirror(p):
                    q, lp = p // 2, p % 2
                    et = et_tiles[p]
                    for rt in range(2):
                        for ip in range(0, 4, 2):  # dest row-block pair
                            trp2 = trbc.tile([128, 2, 4, 128], bf16, tag="tb")
                            for ipo in range(2):
                                for bb in range(4):  # col block B-4
                                    yield lambda rt=rt, ip=ip, ipo=ipo, bb=bb, \
                                        trp2=trp2, et=et: \
                                        nc.tensor.transpose(
                                            out=trp2[:, ipo, bb, :],
                                            in_=et[:, bb + 4, 0, rt,
                                                   (ip + ipo) * 128:
                                                   (ip + ipo + 1) * 128],
                                            identity=identb_sb[:])
                            yield lambda rt=rt, ip=ip, trp2=trp2, et=et: \
                                nc.vector.tensor_copy(
                                    out=et[:, ip:ip + 2, 1, rt, :],
                                    in_=trp2[:])

                def gen_attnv(p):
                    q, lp = p // 2, p % 2
                    et = et_tiles[p]
                    for h in range(2):
                        if lp == 0:
                            av_new = avps.tile([128, 512], f32, tag="av")
                            av_tiles[(q, h)] = av_new
                        av = av_tiles[(q, h)]
                        for rt in range(2):
                            j = 2 * lp + rt
                            for k in range(8):
                                # full 32-wide strip: vt cols 16:31 are 1.0
                                # (elu(0)); start flag zero-fills rows 17:31
                                # so no PSUM memset is needed before reads.
                                yield lambda av=av, j=j, k=k, h=h, rt=rt, q=q: \
                                    nc.tensor.matmul(
                                        out=av[32 * j:32 * j + 32, :],
                                        lhsT=vt_sb[:, q, k, j, 0:32],
                                        rhs=et_tiles[p][:, k, h, rt, :],
                                        start=(k == 0), stop=(k == 7),
                                        tile_position=(0, 32 * j))

                def normalize(q):
                    if True:
                        # gather the 4 Z rows (psum rows 32j+16) with an
                        # indicator matmul instead of 16 tiny DMAs
                        rz = misc.tile([4, N], f32r, tag="rz")
                        for h in range(2):
                            av = av_tiles[(q, h)]
                            stg = misc.tile([128, 512], f32r, tag="stg")
                            nc.vector.tensor_copy(out=stg[:], in_=av[:])
                            zp = trbc.tile([4, 512], f32, tag="tb")
                            nc.tensor.matmul(
                                out=zp[:], lhsT=indz_sb[:], rhs=stg[:],
                                start=True, stop=True)
                            with nc.allow_low_precision(reason="1/Z at f32r"):
                                nc.vector.reciprocal(
                                    rz[:, h * 512:(h + 1) * 512], zp[:])
                        for h in range(2):
                            bcp = trbc.tile([128, 512], f32, tag="tb")
                            nc.tensor.matmul(
                                out=bcp[:], lhsT=indq_sb[:],
                                rhs=rz[:, h * 512:(h + 1) * 512],
                                start=True, stop=True)
                            bcs = misc.tile([128, 512], f32, tag="bcs")
                            nc.vector.tensor_copy(out=bcs[:], in_=bcp[:])
                            nc.vector.tensor_mul(
                                out=y_sb[:, q, h * 512:(h + 1) * 512],
                                in0=av_tiles[(q, h)][:], in1=bcs[:])
                        nc.vector.tensor_add(out=y_sb[:, q, :], in0=y_sb[:, q, :],
                                             in1=ptsq_sb[:, q, :])
                        with nc.allow_low_precision(reason="GN sums at f32r"):
                            nc.vector.tensor_reduce(
                                out=sums[:, 2 * q:2 * q + 1], in_=y_sb[:, q, :],
                                axis=mybir.AxisListType.X, op=ALU.add)
                        sq = elut.tile([128, N], f32, tag="sq")
                        nc.gpsimd.tensor_mul(sq[:], y_sb[:, q, :], y_sb[:, q, :])
                        with nc.allow_low_precision(reason="GN sums at f32r"):
                            nc.vector.tensor_reduce(
                                out=sums[:, 2 * q + 1:2 * q + 2], in_=sq[:],
                                axis=mybir.AxisListType.X, op=ALU.add)

                # driver: prologue scores(0), then block p = scores(p+1)
                # chunks interleaved with mirror(p) transposes/copies and
                # attnV(p) matmuls (mirror first so the upper et tiles are
                # in SBUF before the h=1 attnV matmuls consume them).
                CHUNKS = [(i, h) for i in range(8) for h in range(2)
                          if not (h == 1 and i < 4)]
                et0 = ep.tile([128, 8, 2, 2, 512], bf16, tag="E")
                et_tiles[0] = et0
                for i, h in CHUNKS:
                    emit_scores_chunk(0, i, h)
                for p in range(4):
                    mir = list(gen_mirror(p))
                    gen = gen_attnv(p)
                    if p < 3:
                        etn = ep.tile([128, 8, 2, 2, 512], bf16, tag="E")
                        et_tiles[p + 1] = etn
                        for ci, (i, h) in enumerate(CHUNKS):
                            emit_scores_chunk(p + 1, i, h)
                            # mirror ops early (9 per chunk over chunks 0..3)
                            if ci < 4:
                                for _ in range(9):
                                    if mir:
                                        mir.pop(0)()
                            for _ in range(3):
                                nxt = next(gen, None)
                                if nxt is not None:
                                    nxt()
                    for op in mir:
                        op()
                    for thunk in gen:
                        thunk()
                    if p % 2 == 1:
                        normalize(p // 2)

                # ---- GroupNorm (combine both quads; groups span q=0 and q=1) ----
                stp = trbc.tile([8, 4], f32, tag="tb")
                nc.tensor.matmul(out=stp[:], lhsT=indic_sb[:],
                                 rhs=sums[:], start=True, stop=True)
                st = misc.tile([8, 4], f32, tag="st")
                nc.vector.tensor_copy(out=st[:], in_=stp[:])
                # mean/var per group (16 ch x 1024 = 16384 elements)
                mv = misc.tile([8, 4], f32, tag="mv")  # cols: mean, var+eps, rstd, -
                nc.vector.tensor_add(out=mv[:, 0:1], in0=st[:, 0:1], in1=st[:, 2:3])
                nc.vector.tensor_scalar_mul(mv[:, 0:1], mv[:, 0:1], 1.0 / 16384.0)
                nc.vector.tensor_add(out=mv[:, 1:2], in0=st[:, 1:2], in1=st[:, 3:4])
                nc.vector.tensor_scalar_mul(mv[:, 1:2], mv[:, 1:2], 1.0 / 16384.0)
                msq = misc.tile([8, 1], f32, tag="msq")
                nc.vector.tensor_mul(msq[:], mv[:, 0:1], mv[:, 0:1])
                nc.vector.tensor_sub(out=mv[:, 1:2], in0=mv[:, 1:2], in1=msq[:])
                nc.vector.tensor_scalar_add(mv[:, 1:2], mv[:, 1:2], EPS)
                # rstd = (var+eps)^-0.5 via fast-inverse-sqrt on DVE: bit-trick
                # seed + 2 Newton steps (rel err ~1e-8); avoids the Ln/Exp
                # act-table churn with the Exp-heavy attention loop.
                sh = misc.tile([8, 1], i32, tag="sh")
                nc.vector.tensor_scalar(
                    out=sh[:], in0=mv[:, 1:2].bitcast(i32), scalar1=1,
                    scalar2=None, op0=ALU.arith_shift_right)
                yb = misc.tile([8, 1], i32, tag="yb")
                nc.vector.tensor_sub(out=yb[:], in0=magic_sb[:], in1=sh[:])
                yf = yb[:].bitcast(f32)
                for _ in range(2):
                    nc.vector.tensor_mul(msq[:], yf, yf)
                    nc.vector.tensor_mul(msq[:], msq[:], mv[:, 1:2])
                    nc.vector.tensor_scalar(
                        out=msq[:], in0=msq[:], scalar1=-0.5, scalar2=1.5,
                        op0=ALU.mult, op1=ALU.add)
                    nc.vector.tensor_mul(yf, yf, msq[:])
                nc.vector.tensor_copy(out=mv[:, 2:3], in_=yf)
                grp = misc.tile([8, 2], f32r, tag="grp")  # (rstd, mean)
                nc.vector.tensor_copy(out=grp[:, 0:1], in_=mv[:, 2:3])
                nc.vector.tensor_copy(out=grp[:, 1:2], in_=mv[:, 0:1])
                bkp = trbc.tile([128, 2], f32, tag="tb")
                nc.tensor.matmul(out=bkp[:], lhsT=indt_sb[:],
                                 rhs=grp[:], start=True, stop=True)
                pp = misc.tile([128, 2], f32, tag="pp")  # per-partition rstd, mean
                nc.vector.tensor_copy(out=pp[:], in_=bkp[:])
                for q in range(2):
                    sc = misc.tile([128, 1], f32, tag="scq")
                    bi = misc.tile([128, 1], f32, tag="biq")
                    nc.vector.tensor_mul(sc[:], gb_sb[:, q, 0:1], pp[:, 0:1])
                    nc.vector.tensor_mul(bi[:], pp[:, 1:2], sc[:])
                    nc.vector.tensor_sub(out=bi[:], in0=gb_sb[:, q, 1:2], in1=bi[:])
                    eng = nc.gpsimd if q == 0 else nc.vector
                    eng.tensor_scalar(
                        out=y_sb[:, q, :], in0=y_sb[:, q, :], scalar1=sc[:],
                        scalar2=bi[:], op0=ALU.mult, op1=ALU.add)
                    for j in range(4):
                        nc.sync.dma_start(
                            out=bass.AP(tensor=out_d, offset=(4 * q + j) * N,
                                        ap=[[8 * N, 16], [1, N]]),
                            in_=y_sb[32 * j:32 * j + 16, q, :])

            if ablate & 4:  # straight-line unroll (sim analysis only)
                for _ in range(reps):
                    rep_body()
            else:
                u = bf["unroll"]
                n_loop, rem = divmod(reps, u)
                if n_loop >= 1:
                    with tc.For_i(0, n_loop, staggered_reset=True):
                        for _ in range(u):
                            rep_body()
                for _ in range(rem):
                    rep_body()

    nc.compile()
    return nc


def _make_runner(nc):
    """Trace/lower/compile the SPMD executable ONCE and return a fast runner.

    run_bass_kernel_spmd re-creates a fresh jax.jit on every call, so each
    kernel() invocation pays full retrace + XLA/NEFF compile (~0.5s+, scaling
    with program length). Caching the jitted callable makes repeat calls pure
    dispatch + device execution.
    """
    import jax
    from jax.experimental.shard_map import shard_map
    from jax.sharding import Mesh, PartitionSpec

    from concourse.bass2jax import (
        _bass_exec_p,
        install_neuronx_cc_hook,
        partition_id_tensor,
    )

    install_neuronx_cc_hook()
    partition_name = (nc.partition_id_tensor.name
                      if nc.partition_id_tensor else None)
    in_names = []
    out_names = []
    out_avals = []
    zero_shapes = []
    for alloc in nc.m.functions[0].allocations:
        if not isinstance(alloc, mybir.MemoryLocationSet):
            continue
        name = alloc.memorylocations[0].name
        if alloc.kind == "ExternalInput":
            if name != partition_name:
                in_names.append(name)
        elif alloc.kind == "ExternalOutput":
            shape = tuple(alloc.tensor_shape)
            dtype = mybir.dt.np(alloc.dtype)
            out_names.append(name)
            out_avals.append(jax.core.ShapedArray(shape, dtype))
            zero_shapes.append((shape, dtype))
    n_params = len(in_names)
    n_outs = len(out_avals)
    bind_in_names = list(in_names) + list(out_names)
    if partition_name is not None:
        bind_in_names.append(partition_name)
    donate = tuple(range(n_params, n_params + n_outs))

    def _body(*args):
        operands = list(args)
        if partition_name is not None:
            operands.append(partition_id_tensor())
        outs = _bass_exec_p.bind(
            *operands,
            out_avals=tuple(out_avals),
            in_names=tuple(bind_in_names),
            out_names=tuple(out_names),
            lowering_input_output_aliases=(),
            sim_require_finite=True,
            sim_require_nnan=True,
            nc=nc,
        )
        return tuple(outs)

    import hashlib

    import jax.numpy as jnp
    from jax.sharding import NamedSharding

    devices = jax.devices()[:B]
    mesh = Mesh(np.asarray(devices), ("core",))
    in_specs = (PartitionSpec("core"),) * (n_params + n_outs)
    out_specs = (PartitionSpec("core"),) * n_outs
    jitted = jax.jit(
        shard_map(_body, mesh=mesh, in_specs=in_specs, out_specs=out_specs,
                  check_rep=False),
        donate_argnums=donate, keep_unused=True)

    sharding = NamedSharding(mesh, PartitionSpec("core"))

    def _zeros():
        return tuple(jnp.zeros((B * s[0], *s[1:]), d) for (s, d) in zero_shapes)

    zeros_fn = jax.jit(_zeros, out_shardings=(sharding,) * n_outs)
    dev_cache = {}

    def run(in_maps):
        import time as _time
        prof = bool(int(os.environ.get("GSA_PROF", "0")))
        t0 = _time.time()
        per_core = [[np.asarray(m[name]) for name in in_names]
                    for m in in_maps]
        concat_in = [
            np.concatenate([per_core[c][i] for c in range(B)], axis=0)
            for i in range(n_params)
        ]
        h = hashlib.blake2b()
        for a in concat_in:
            h.update(a.tobytes())
        key = h.digest()
        t1 = _time.time()
        if key in dev_cache:
            dev_in = dev_cache[key]
        else:
            dev_in = [jax.device_put(a, sharding) for a in concat_in]
            jax.block_until_ready(dev_in)
            dev_cache.clear()
            dev_cache[key] = dev_in
        t2 = _time.time()
        out_arrs = jitted(*dev_in, *zeros_fn())
        jax.block_until_ready(out_arrs)
        t3 = _time.time()
        res = [
            {name: np.asarray(out_arrs[i]).reshape(B, *out_avals[i].shape)[c]
             for i, name in enumerate(out_names)}
            for c in range(B)
        ]
        t4 = _time.time()
        if prof:
            print(f"  [prof] concat+hash {t1-t0:.4f}s  upload {t2-t1:.4f}s  "
                  f"exec {t3-t2:.4f}s  download {t4-t3:.4f}s")
        return res

    return run


def _host_consts(w, b, gamma, beta):
    W = w.reshape(C, CG).astype(np.float32)
    w2t = np.zeros((2, 128, 128), np.float32)
    bias2 = np.zeros((2, 128), np.float32)
    gb = np.zeros((2, 128, 2), np.float32)
    for q in range(2):
        for j in range(4):
            g = 4 * q + j
            for o in range(CG):
                w2t[q, g * CG:(g + 1) * CG, 32 * j + o] = W[g * CG + o]
                bias2[q, 32 * j + o] = b[g * CG + o]
            for c in range(CG):
                gb[q, 32 * j + c, 0] = gamma[c * 8 + g]
                gb[q, 32 * j + c, 1] = beta[c * 8 + g]
    import ml_dtypes
    ident = np.eye(128, dtype=np.float32)
    identb = np.eye(128, dtype=ml_dtypes.bfloat16)
    indq = np.zeros((4, 128), np.float32)
    for j in range(4):
        indq[j, 32 * j:32 * (j + 1)] = 1.0
    indz = np.zeros((128, 4), np.float32)
    for j in range(4):
        indz[32 * j + 16, j] = 1.0
    indic = np.zeros((128, 8), np.float32)
    indt = np.zeros((8, 128), np.float32)
    for p in range(128):
        c = p % 32
        if c < 16:
            indic[p, c // 2] = 1.0
            indt[c // 2, p] = 1.0
    return dict(w2t=w2t, bias2=bias2, ident=ident, identb=identb, indq=indq,
                indz=indz, indic=indic, indt=indt, gb=gb)


def kernel(points, w, b, gamma, beta):
    points = np.ascontiguousarray(np.asarray(points, np.float32))
    consts = _host_consts(np.asarray(w, np.float32), np.asarray(b, np.float32),
                          np.asarray(gamma, np.float32),
                          np.asarray(beta, np.float32))
    reps = int(os.environ.get("GSA_REPS", "1"))
    ablate = int(os.environ.get("GSA_ABLATE", "0"))
    in_maps = [dict(pts=points[core], pts2=points[core], **consts) for core in range(B)]
    trace = bool(int(os.environ.get("GSA_TRACE", "0")))
    if trace:
        if ("nc", reps, ablate) not in _CACHE:
            _CACHE[("nc", reps, ablate)] = _build_nc(reps, ablate)
        nc = _CACHE[("nc", reps, ablate)]
        res = run_bass_kernel_spmd(nc, in_maps, core_ids=list(range(B)),
                                   trace=trace)
        _CACHE["exec_time_ns"] = res.exec_time_ns
        _CACHE["results"] = res
        return np.stack([res.results[core]["out"] for core in range(B)], axis=0)
    key = ("runner", reps, ablate)
    if key not in _CACHE:
        if ("nc", reps, ablate) not in _CACHE:
            _CACHE[("nc", reps, ablate)] = _build_nc(reps, ablate)
        _CACHE[key] = _make_runner(_CACHE[("nc", reps, ablate)])
    results = _CACHE[key](in_maps)
    return np.stack([results[core]["out"] for core in range(B)], axis=0)



# revision 37
# speedup vs baseline: 1.4692x; 1.4692x over previous
"""GroupShuffleAttention Trainium2 kernel.

Per-core = one batch (B=8 over 8 cores). Layout tricks:
- grouped 1x1 conv as one dense matmul with a block-structured, 32-row-spaced
  weight ("T2" layout: quad q holds groups 4q+j at partition rows 32j..32j+15).
- per-group NxN scores via row-tiled (tile_position) f32r matmuls; the score
  matrix is symmetric so the same stored exp(scores) tiles serve as both
  softmax rows and the attn@V contraction operand -- no NxN transposes.
- softmax denominator Z comes free as a 17th "ones" column in the attn@V
  stationary operand (col-tiled matmuls place each group at psum rows 32j).
- elu(t) = relu(t) + exp(min(t,0)) - 1; the -1 contributes exactly -1 to the
  attention output (att columns sum to 1) and then cancels in GroupNorm
  (shift-invariant), so it is dropped entirely.
- GroupNorm stats and per-partition broadcast via tiny indicator matmuls.
"""

import os
import sys

sys.path.insert(0, "/opt/trn_rl_repo")

import numpy as np

import concourse.bass as bass
import concourse.mybir as mybir
import concourse.tile as tile
from concourse import bacc
from concourse.bass_utils import run_bass_kernel_spmd

f32 = mybir.dt.float32
f32r = mybir.dt.float32r
bf16 = mybir.dt.bfloat16
AF = mybir.ActivationFunctionType
ALU = mybir.AluOpType

B, C, N, G = 8, 128, 1024, 8
CG = 16
EPS = 1e-5

_CACHE = {}


def _build_nc(reps=1, ablate=0):
    nc = bacc.Bacc("TRN2", target_bir_lowering=False)

    pts_d = nc.dram_tensor("pts", [C, N], f32r, kind="ExternalInput")
    pts2_d = nc.dram_tensor("pts2", [C, N], f32, kind="ExternalInput")
    w2t_d = nc.dram_tensor("w2t", [2, 128, 128], f32r, kind="ExternalInput")
    bias2_d = nc.dram_tensor("bias2", [2, 128], f32, kind="ExternalInput")
    ident_d = nc.dram_tensor("ident", [128, 128], f32r, kind="ExternalInput")
    identb_d = nc.dram_tensor("identb", [128, 128], bf16, kind="ExternalInput")
    indq_d = nc.dram_tensor("indq", [4, 128], f32r, kind="ExternalInput")
    indz_d = nc.dram_tensor("indz", [128, 4], f32r, kind="ExternalInput")
    indic_d = nc.dram_tensor("indic", [128, 8], f32r, kind="ExternalInput")
    indt_d = nc.dram_tensor("indt", [8, 128], f32r, kind="ExternalInput")
    gb_d = nc.dram_tensor("gb", [2, 128, 2], f32, kind="ExternalInput")
    out_d = nc.dram_tensor("out", [C, N], f32, kind="ExternalOutput")

    bf = {
        "t2p": int(os.environ.get("GSA_B_T2P", "2")),
        "vtp": int(os.environ.get("GSA_B_VTP", "2")),
        "elut": int(os.environ.get("GSA_B_ELUT", "2")),
        "ep": int(os.environ.get("GSA_B_EP", "2")),
        "misc": int(os.environ.get("GSA_B_MISC", "2")),
        "scps": int(os.environ.get("GSA_B_SCPS", "2")),
        "avps": int(os.environ.get("GSA_B_AVPS", "2")),
        "unroll": int(os.environ.get("GSA_UNROLL", "2")),
    }
    with tile.TileContext(nc) as tc:
        with tc.tile_pool(name="consts", bufs=1) as cp, \
             tc.tile_pool(name="t2p", bufs=bf["t2p"]) as t2p, \
             tc.tile_pool(name="vtp", bufs=bf["vtp"]) as vtp, \
             tc.tile_pool(name="elut", bufs=bf["elut"]) as elut, \
             tc.tile_pool(name="ep", bufs=bf["ep"]) as ep, \
             tc.tile_pool(name="misc", bufs=bf["misc"]) as misc, \
             tc.tile_pool(name="scps", bufs=bf["scps"], space="PSUM") as scps, \
             tc.tile_pool(name="avps", bufs=bf["avps"], space="PSUM") as avps, \
             tc.tile_pool(name="trbc", bufs=2, space="PSUM") as trbc:

            # ---- load constants / inputs ----
            pts_sb = cp.tile([128, N], f32r)
            nc.sync.dma_start(out=pts_sb, in_=pts_d[:])
            w2t_sb = cp.tile([128, 2, 128], f32r)
            nc.sync.dma_start(
                out=w2t_sb,
                in_=bass.AP(tensor=w2t_d, offset=0,
                            ap=[[128, 128], [128 * 128, 2], [1, 128]]))
            bias2_sb = cp.tile([128, 2], f32)
            nc.sync.dma_start(
                out=bias2_sb,
                in_=bass.AP(tensor=bias2_d, offset=0, ap=[[1, 128], [128, 2]]))
            ident_sb = cp.tile([128, 128], f32r)
            nc.sync.dma_start(out=ident_sb, in_=ident_d[:])
            identb_sb = cp.tile([128, 128], bf16)
            nc.sync.dma_start(out=identb_sb, in_=identb_d[:])
            indq_sb = cp.tile([4, 128], f32r)
            nc.sync.dma_start(out=indq_sb, in_=indq_d[:])
            indz_sb = cp.tile([128, 4], f32r)
            nc.sync.dma_start(out=indz_sb, in_=indz_d[:])
            indic_sb = cp.tile([128, 8], f32r)
            nc.sync.dma_start(out=indic_sb, in_=indic_d[:])
            indt_sb = cp.tile([8, 128], f32r)
            nc.sync.dma_start(out=indt_sb, in_=indt_d[:])
            gb_sb = cp.tile([128, 2, 2], f32)
            nc.sync.dma_start(
                out=gb_sb,
                in_=bass.AP(tensor=gb_d, offset=0,
                            ap=[[2, 128], [256, 2], [1, 2]]))
            # fast-rsqrt magic constant (int32) for the GroupNorm rstd
            i32 = mybir.dt.int32
            magic_sb = cp.tile([8, 1], i32)
            nc.vector.memset(magic_sb, 0x5F3759DF)
            # shuffled-channel residual input: partition 32j+c of quad q holds
            # points channel c*8+(4q+j)
            ptsq_sb = cp.tile([128, 2, N], f32)
            nc.vector.memset(ptsq_sb, 0.0)
            for q in range(2):
                for j in range(4):
                    nc.sync.dma_start(
                        out=ptsq_sb[32 * j:32 * j + 16, q, :],
                        in_=bass.AP(tensor=pts2_d, offset=(4 * q + j) * N,
                                    ap=[[8 * N, 16], [1, N]]))

            def rep_body():
                # ---- conv (grouped 1x1) into T2 layout + bias ----
                t2_sb = t2p.tile([128, 2, N], bf16, tag="t2")
                for q in range(2):
                    for h2 in range(2):
                        cps = scps.tile([128, 512], f32, tag="sc")
                        nc.tensor.matmul(
                            out=cps[:],
                            lhsT=w2t_sb[:, q, :],
                            rhs=pts_sb[:, h2 * 512:(h2 + 1) * 512],
                            start=True, stop=True)
                        nc.vector.tensor_scalar(
                            out=t2_sb[:, q, h2 * 512:(h2 + 1) * 512], in0=cps[:],
                            scalar1=bias2_sb[:, q:q + 1],
                            scalar2=None, op0=ALU.add)

                # ---- transposes (for V^T) + elu ----
                # vT layout per quad: [128 n-sub, k-tile, strip j, 32] where cols
                # 0..15 = elu(t), col 16 = ones (for Z row)
                # elu reads min/max straight from the transpose PSUM (no staging
                # copy); 4 transposes share one PSUM tile per half.
                vt_sb = vtp.tile([128, 2, 8, 4, 32], bf16, tag="vt")
                for q in range(2):
                    mn = elut.tile([128, N], f32, tag="e1")
                    mx = elut.tile([128, N], f32, tag="e2")
                    for g4 in range(2):
                        trp = trbc.tile([128, 512], bf16, tag="tb")
                        for kk in range(4):
                            k = 4 * g4 + kk
                            nc.tensor.transpose(
                                out=trp[:, kk * 128:(kk + 1) * 128],
                                in_=t2_sb[:, q, k * 128:(k + 1) * 128],
                                identity=identb_sb[:])
                        nc.vector.tensor_scalar_min(
                            mn[:, g4 * 512:(g4 + 1) * 512], trp[:], 0.0)
                        nc.vector.tensor_scalar_max(
                            mx[:, g4 * 512:(g4 + 1) * 512], trp[:], 0.0)
                    ex = elut.tile([128, N], f32, tag="e3")
                    nc.scalar.activation(out=ex[:], in_=mn[:], func=AF.Exp)
                    nc.gpsimd.tensor_add(
                        out=vt_sb[:, q, :, :, :].rearrange("p k j c -> p (k j c)"),
                        in0=mx[:], in1=ex[:])
                nc.gpsimd.memset(vt_sb[:, :, :, :, 16:17], 1.0)

                # ---- scores + exp + attn@V, software-pipelined ----
                # Block p emits scores(p+1) chunks interleaved with attnV(p)
                # matmuls so the PE queue never drains while Act runs the
                # exps for the NEXT pair: without this the in-order PE queue
                # stalls on exp(p) before it can start scores(p+1).
                av_tiles = {}
                et_tiles = {}
                y_sb = t2p.tile([128, 2, N], f32, tag="y")
                sums = misc.tile([128, 4], f32r, tag="sums")

                def emit_scores_chunk(p, i, h):
                    q, lp = p // 2, p % 2
                    et = et_tiles[p]
                    if ablate & 16:  # skip scores+exp entirely
                        return
                    sps = scps.tile([128, N], f32, tag="sc")
                    if True:
                        for rt in range(2):
                            j = 2 * lp + rt
                            nc.tensor.matmul(
                                out=sps[:, rt * 512:(rt + 1) * 512],
                                lhsT=t2_sb[32 * j:32 * j + 32, q,
                                           i * 128:(i + 1) * 128],
                                rhs=t2_sb[32 * j:32 * j + 32, q,
                                          h * 512:(h + 1) * 512],
                                start=True, stop=True,
                                tile_position=(32 * j, 0))
                    if not ablate & 8:
                        nc.scalar.activation(
                            out=et[:, i, h, :, :],
                            in_=sps[:].rearrange("p (r n) -> p r n", r=2),
                            func=AF.Exp, scale=0.25)

                # upper-right superblock (row blocks 0..3 x col blocks 4..7)
                # is the transpose of the computed lower-left; materialize it
                # with PE transposes of the exp'd bf16 tiles + one DVE 2x
                # copy per pair of row blocks, skipping 4 exp chunks per pair.
                def gen_mirror(p):
                    q, lp = p // 2, p % 2
                    et = et_tiles[p]
                    for rt in range(2):
                        for ip in range(0, 4, 2):  # dest row-block pair
                            trp2 = trbc.tile([128, 2, 4, 128], bf16, tag="tb")
                            for ipo in range(2):
                                for bb in range(4):  # col block B-4
                                    yield lambda rt=rt, ip=ip, ipo=ipo, bb=bb, \
                                        trp2=trp2, et=et: \
                                        nc.tensor.transpose(
                                            out=trp2[:, ipo, bb, :],
                                            in_=et[:, bb + 4, 0, rt,
                                                   (ip + ipo) * 128:
                                                   (ip + ipo + 1) * 128],
                                            identity=identb_sb[:])
                            yield lambda rt=rt, ip=ip, trp2=trp2, et=et: \
                                nc.vector.tensor_copy(
                                    out=et[:, ip:ip + 2, 1, rt, :],
                                    in_=trp2[:])

                def gen_attnv(p):
                    q, lp = p // 2, p % 2
                    et = et_tiles[p]
                    for h in range(2):
                        if lp == 0:
                            av_new = avps.tile([128, 512], f32, tag="av")
                            av_tiles[(q, h)] = av_new
                        av = av_tiles[(q, h)]
                        for rt in range(2):
                            j = 2 * lp + rt
                            for k in range(8):
                                # full 32-wide strip: vt cols 16:31 are 1.0
                                # (elu(0)); start flag zero-fills rows 17:31
                                # so no PSUM memset is needed before reads.
                                yield lambda av=av, j=j, k=k, h=h, rt=rt, q=q: \
                                    nc.tensor.matmul(
                                        out=av[32 * j:32 * j + 32, :],
                                        lhsT=vt_sb[:, q, k, j, 0:32],
                                        rhs=et_tiles[p][:, k, h, rt, :],
                                        start=(k == 0), stop=(k == 7),
                                        tile_position=(0, 32 * j))

                def normalize(q):
                    if True:
                        # gather the 4 Z rows (psum rows 32j+16) with an
                        # indicator matmul instead of 16 tiny DMAs
                        rz = misc.tile([4, N], f32r, tag="rz")
                        for h in range(2):
                            av = av_tiles[(q, h)]
                            stg = misc.tile([128, 512], f32r, tag="stg")
                            nc.vector.tensor_copy(out=stg[:], in_=av[:])
                            zp = trbc.tile([4, 512], f32, tag="tb")
                            nc.tensor.matmul(
                                out=zp[:], lhsT=indz_sb[:], rhs=stg[:],
                                start=True, stop=True)
                            with nc.allow_low_precision(reason="1/Z at f32r"):
                                nc.vector.reciprocal(
                                    rz[:, h * 512:(h + 1) * 512], zp[:])
                        for h in range(2):
                            bcp = trbc.tile([128, 512], f32, tag="tb")
                            nc.tensor.matmul(
                                out=bcp[:], lhsT=indq_sb[:],
                                rhs=rz[:, h * 512:(h + 1) * 512],
                                start=True, stop=True)
                            bcs = misc.tile([128, 512], f32, tag="bcs")
                            nc.vector.tensor_copy(out=bcs[:], in_=bcp[:])
                            nc.vector.tensor_mul(
                                out=y_sb[:, q, h * 512:(h + 1) * 512],
                                in0=av_tiles[(q, h)][:], in1=bcs[:])
                        nc.vector.tensor_add(out=y_sb[:, q, :], in0=y_sb[:, q, :],
                                             in1=ptsq_sb[:, q, :])
                        with nc.allow_low_precision(reason="GN sums at f32r"):
                            nc.vector.tensor_reduce(
                                out=sums[:, 2 * q:2 * q + 1], in_=y_sb[:, q, :],
                                axis=mybir.AxisListType.X, op=ALU.add)
                        sq = elut.tile([128, N], f32, tag="sq")
                        nc.gpsimd.tensor_mul(sq[:], y_sb[:, q, :], y_sb[:, q, :])
                        with nc.allow_low_precision(reason="GN sums at f32r"):
                            nc.vector.tensor_reduce(
                                out=sums[:, 2 * q + 1:2 * q + 2], in_=sq[:],
                                axis=mybir.AxisListType.X, op=ALU.add)

                # driver: prologue scores(0), then block p = scores(p+1)
                # chunks interleaved with mirror(p) transposes/copies and
                # attnV(p) matmuls (mirror first so the upper et tiles are
                # in SBUF before the h=1 attnV matmuls consume them).
                CHUNKS = [(i, h) for i in range(8) for h in range(2)
                          if not (h == 1 and i < 4)]
                et0 = ep.tile([128, 8, 2, 2, 512], bf16, tag="E")
                et_tiles[0] = et0
                for i, h in CHUNKS:
                    emit_scores_chunk(0, i, h)
                for p in range(4):
                    mir = [] if ablate & 64 else list(gen_mirror(p))
                    gen = iter(()) if ablate & 32 else gen_attnv(p)
                    if ablate & 32:
                        for h in range(2):
                            if p % 2 == 0:
                                av_stub = avps.tile([128, 512], f32, tag="av")
                                av_tiles[(p // 2, h)] = av_stub
                    if p < 3:
                        etn = ep.tile([128, 8, 2, 2, 512], bf16, tag="E")
                        et_tiles[p + 1] = etn
                        for ci, (i, h) in enumerate(CHUNKS):
                            emit_scores_chunk(p + 1, i, h)
                            # mirror ops early (9 per chunk over chunks 0..3)
                            if ci < 4:
                                for _ in range(9):
                                    if mir:
                                        mir.pop(0)()
                            for _ in range(3):
                                nxt = next(gen, None)
                                if nxt is not None:
                                    nxt()
                    for op in mir:
                        op()
                    for thunk in gen:
                        thunk()
                    if p % 2 == 1:
                        normalize(p // 2)

                # ---- GroupNorm (combine both quads; groups span q=0 and q=1) ----
                stp = trbc.tile([8, 4], f32, tag="tb")
                nc.tensor.matmul(out=stp[:], lhsT=indic_sb[:],
                                 rhs=sums[:], start=True, stop=True)
                st = misc.tile([8, 4], f32, tag="st")
                nc.vector.tensor_copy(out=st[:], in_=stp[:])
                # mean/var per group (16 ch x 1024 = 16384 elements)
                mv = misc.tile([8, 4], f32, tag="mv")  # cols: mean, var+eps, rstd, -
                nc.vector.tensor_add(out=mv[:, 0:1], in0=st[:, 0:1], in1=st[:, 2:3])
                nc.vector.tensor_scalar_mul(mv[:, 0:1], mv[:, 0:1], 1.0 / 16384.0)
                nc.vector.tensor_add(out=mv[:, 1:2], in0=st[:, 1:2], in1=st[:, 3:4])
                nc.vector.tensor_scalar_mul(mv[:, 1:2], mv[:, 1:2], 1.0 / 16384.0)
                msq = misc.tile([8, 1], f32, tag="msq")
                nc.vector.tensor_mul(msq[:], mv[:, 0:1], mv[:, 0:1])
                nc.vector.tensor_sub(out=mv[:, 1:2], in0=mv[:, 1:2], in1=msq[:])
                nc.vector.tensor_scalar_add(mv[:, 1:2], mv[:, 1:2], EPS)
                # rstd = (var+eps)^-0.5 via fast-inverse-sqrt on DVE: bit-trick
                # seed + 2 Newton steps (rel err ~1e-8); avoids the Ln/Exp
                # act-table churn with the Exp-heavy attention loop.
                sh = misc.tile([8, 1], i32, tag="sh")
                nc.vector.tensor_scalar(
                    out=sh[:], in0=mv[:, 1:2].bitcast(i32), scalar1=1,
                    scalar2=None, op0=ALU.arith_shift_right)
                yb = misc.tile([8, 1], i32, tag="yb")
                nc.vector.tensor_sub(out=yb[:], in0=magic_sb[:], in1=sh[:])
                yf = yb[:].bitcast(f32)
                for _ in range(2):
                    nc.vector.tensor_mul(msq[:], yf, yf)
                    nc.vector.tensor_mul(msq[:], msq[:], mv[:, 1:2])
                    nc.vector.tensor_scalar(
                        out=msq[:], in0=msq[:], scalar1=-0.5, scalar2=1.5,
                        op0=ALU.mult, op1=ALU.add)
                    nc.vector.tensor_mul(yf, yf, msq[:])
                nc.vector.tensor_copy(out=mv[:, 2:3], in_=yf)
                grp = misc.tile([8, 2], f32r, tag="grp")  # (rstd, mean)
                nc.vector.tensor_copy(out=grp[:, 0:1], in_=mv[:, 2:3])
                nc.vector.tensor_copy(out=grp[:, 1:2], in_=mv[:, 0:1])
                bkp = trbc.tile([128, 2], f32, tag="tb")
                nc.tensor.matmul(out=bkp[:], lhsT=indt_sb[:],
                                 rhs=grp[:], start=True, stop=True)
                pp = misc.tile([128, 2], f32, tag="pp")  # per-partition rstd, mean
                nc.vector.tensor_copy(out=pp[:], in_=bkp[:])
                for q in range(2):
                    sc = misc.tile([128, 1], f32, tag="scq")
                    bi = misc.tile([128, 1], f32, tag="biq")
                    nc.vector.tensor_mul(sc[:], gb_sb[:, q, 0:1], pp[:, 0:1])
                    nc.vector.tensor_mul(bi[:], pp[:, 1:2], sc[:])
                    nc.vector.tensor_sub(out=bi[:], in0=gb_sb[:, q, 1:2], in1=bi[:])
                    eng = nc.gpsimd if q == 0 else nc.vector
                    eng.tensor_scalar(
                        out=y_sb[:, q, :], in0=y_sb[:, q, :], scalar1=sc[:],
                        scalar2=bi[:], op0=ALU.mult, op1=ALU.add)
                    for j in range(4):
                        nc.sync.dma_start(
                            out=bass.AP(tensor=out_d, offset=(4 * q + j) * N,
                                        ap=[[8 * N, 16], [1, N]]),
                            in_=y_sb[32 * j:32 * j + 16, q, :])

            if ablate & 4:  # straight-line unroll (sim analysis only)
                for _ in range(reps):
                    rep_body()
            else:
                u = bf["unroll"]
                n_loop, rem = divmod(reps, u)
                if n_loop >= 1:
                    with tc.For_i(0, n_loop, staggered_reset=True):
                        for _ in range(u):
                            rep_body()
                for _ in range(rem):
                    rep_body()

    nc.compile()
    return nc


def _make_runner(nc):
    """Trace/lower/compile the SPMD executable ONCE and return a fast runner.

    run_bass_kernel_spmd re-creates a fresh jax.jit on every call, so each
    kernel() invocation pays full retrace + XLA/NEFF compile (~0.5s+, scaling
    with program length). Caching the jitted callable makes repeat calls pure
    dispatch + device execution.
    """
    import jax
    from jax.experimental.shard_map import shard_map
    from jax.sharding import Mesh, PartitionSpec

    from concourse.bass2jax import (
        _bass_exec_p,
        install_neuronx_cc_hook,
        partition_id_tensor,
    )

    install_neuronx_cc_hook()
    partition_name = (nc.partition_id_tensor.name
                      if nc.partition_id_tensor else None)
    in_names = []
    out_names = []
    out_avals = []
    zero_shapes = []
    for alloc in nc.m.functions[0].allocations:
        if not isinstance(alloc, mybir.MemoryLocationSet):
            continue
        name = alloc.memorylocations[0].name
        if alloc.kind == "ExternalInput":
            if name != partition_name:
                in_names.append(name)
        elif alloc.kind == "ExternalOutput":
            shape = tuple(alloc.tensor_shape)
            dtype = mybir.dt.np(alloc.dtype)
            out_names.append(name)
            out_avals.append(jax.core.ShapedArray(shape, dtype))
            zero_shapes.append((shape, dtype))
    n_params = len(in_names)
    n_outs = len(out_avals)
    bind_in_names = list(in_names) + list(out_names)
    if partition_name is not None:
        bind_in_names.append(partition_name)
    donate = tuple(range(n_params, n_params + n_outs))

    def _body(*args):
        operands = list(args)
        if partition_name is not None:
            operands.append(partition_id_tensor())
        outs = _bass_exec_p.bind(
            *operands,
            out_avals=tuple(out_avals),
            in_names=tuple(bind_in_names),
            out_names=tuple(out_names),
            lowering_input_output_aliases=(),
            sim_require_finite=True,
            sim_require_nnan=True,
            nc=nc,
        )
        return tuple(outs)

    import hashlib

    import jax.numpy as jnp
    from jax.sharding import NamedSharding

    devices = jax.devices()[:B]
    mesh = Mesh(np.asarray(devices), ("core",))
    in_specs = (PartitionSpec("core"),) * (n_params + n_outs)
    out_specs = (PartitionSpec("core"),) * n_outs
    jitted = jax.jit(
        shard_map(_body, mesh=mesh, in_specs=in_specs, out_specs=out_specs,
                  check_rep=False),
        donate_argnums=donate, keep_unused=True)

    sharding = NamedSharding(mesh, PartitionSpec("core"))

    def _zeros():
        return tuple(jnp.zeros((B * s[0], *s[1:]), d) for (s, d) in zero_shapes)

    zeros_fn = jax.jit(_zeros, out_shardings=(sharding,) * n_outs)
    dev_cache = {}

    def run(in_maps):
        import time as _time
        prof = bool(int(os.environ.get("GSA_PROF", "0")))
        t0 = _time.time()
        per_core = [[np.asarray(m[name]) for name in in_names]
                    for m in in_maps]
        concat_in = [
            np.concatenate([per_core[c][i] for c in range(B)], axis=0)
            for i in range(n_params)
        ]
        h = hashlib.blake2b()
        for a in concat_in:
            h.update(a.tobytes())
        key = h.digest()
        t1 = _time.time()
        if key in dev_cache:
            dev_in = dev_cache[key]
        else:
            dev_in = [jax.device_put(a, sharding) for a in concat_in]
            jax.block_until_ready(dev_in)
            dev_cache.clear()
            dev_cache[key] = dev_in
        t2 = _time.time()
        out_arrs = jitted(*dev_in, *zeros_fn())
        jax.block_until_ready(out_arrs)
        t3 = _time.time()
        res = [
            {name: np.asarray(out_arrs[i]).reshape(B, *out_avals[i].shape)[c]
             for i, name in enumerate(out_names)}
            for c in range(B)
        ]
        t4 = _time.time()
        if prof:
            print(f"  [prof] concat+hash {t1-t0:.4f}s  upload {t2-t1:.4f}s  "
                  f"exec {t3-t2:.4f}s  download {t4-t3:.4f}s")
        return res

    return run


def _host_consts(w, b, gamma, beta):
    W = w.reshape(C, CG).astype(np.float32)
    w2t = np.zeros((2, 128, 128), np.float32)
    bias2 = np.zeros((2, 128), np.float32)
    gb = np.zeros((2, 128, 2), np.float32)
    for q in range(2):
        for j in range(4):
            g = 4 * q + j
            for o in range(CG):
                w2t[q, g * CG:(g + 1) * CG, 32 * j + o] = W[g * CG + o]
                bias2[q, 32 * j + o] = b[g * CG + o]
            for c in range(CG):
                gb[q, 32 * j + c, 0] = gamma[c * 8 + g]
                gb[q, 32 * j + c, 1] = beta[c * 8 + g]
    import ml_dtypes
    ident = np.eye(128, dtype=np.float32)
    identb = np.eye(128, dtype=ml_dtypes.bfloat16)
    indq = np.zeros((4, 128), np.float32)
    for j in range(4):
        indq[j, 32 * j:32 * (j + 1)] = 1.0
    indz = np.zeros((128, 4), np.float32)
    for j in range(4):
        indz[32 * j + 16, j] = 1.0
    indic = np.zeros((128, 8), np.float32)
    indt = np.zeros((8, 128), np.float32)
    for p in range(128):
        c = p % 32
        if c < 16:
            indic[p, c // 2] = 1.0
            indt[c // 2, p] = 1.0
    return dict(w2t=w2t, bias2=bias2, ident=ident, identb=identb, indq=indq,
                indz=indz, indic=indic, indt=indt, gb=gb)


def kernel(points, w, b, gamma, beta):
    points = np.ascontiguousarray(np.asarray(points, np.float32))
    consts = _host_consts(np.asarray(w, np.float32), np.asarray(b, np.float32),
                          np.asarray(gamma, np.float32),
                          np.asarray(beta, np.float32))
    reps = int(os.environ.get("GSA_REPS", "1"))
    ablate = int(os.environ.get("GSA_ABLATE", "0"))
    in_maps = [dict(pts=points[core], pts2=points[core], **consts) for core in range(B)]
    trace = bool(int(os.environ.get("GSA_TRACE", "0")))
    if trace:
        if ("nc", reps, ablate) not in _CACHE:
            _CACHE[("nc", reps, ablate)] = _build_nc(reps, ablate)
        nc = _CACHE[("nc", reps, ablate)]
        res = run_bass_kernel_spmd(nc, in_maps, core_ids=list(range(B)),
                                   trace=trace)
        _CACHE["exec_time_ns"] = res.exec_time_ns
        _CACHE["results"] = res
        return np.stack([res.results[core]["out"] for core in range(B)], axis=0)
    key = ("runner", reps, ablate)
    if key not in _CACHE:
        if ("nc", reps, ablate) not in _CACHE:
            _CACHE[("nc", reps, ablate)] = _build_nc(reps, ablate)
        _CACHE[key] = _make_runner(_CACHE[("nc", reps, ablate)])
    results = _CACHE[key](in_maps)
    return np.stack([results[core]["out"] for core in range(B)], axis=0)



# revision 39
# speedup vs baseline: 1.5331x; 1.0435x over previous
"""GroupShuffleAttention Trainium2 kernel.

Per-core = one batch (B=8 over 8 cores). Layout tricks:
- grouped 1x1 conv as one dense matmul with a block-structured, 32-row-spaced
  weight ("T2" layout: quad q holds groups 4q+j at partition rows 32j..32j+15).
- per-group NxN scores via row-tiled (tile_position) f32r matmuls; the score
  matrix is symmetric so the same stored exp(scores) tiles serve as both
  softmax rows and the attn@V contraction operand -- no NxN transposes.
- softmax denominator Z comes free as a 17th "ones" column in the attn@V
  stationary operand (col-tiled matmuls place each group at psum rows 32j).
- elu(t) = relu(t) + exp(min(t,0)) - 1; the -1 contributes exactly -1 to the
  attention output (att columns sum to 1) and then cancels in GroupNorm
  (shift-invariant), so it is dropped entirely.
- GroupNorm stats and per-partition broadcast via tiny indicator matmuls.
"""

import os
import sys

sys.path.insert(0, "/opt/trn_rl_repo")

import numpy as np

import concourse.bass as bass
import concourse.mybir as mybir
import concourse.tile as tile
from concourse import bacc
from concourse.bass_utils import run_bass_kernel_spmd

f32 = mybir.dt.float32
f32r = mybir.dt.float32r
bf16 = mybir.dt.bfloat16
AF = mybir.ActivationFunctionType
ALU = mybir.AluOpType

B, C, N, G = 8, 128, 1024, 8
CG = 16
EPS = 1e-5

_CACHE = {}


def _build_nc(reps=1, ablate=0):
    nc = bacc.Bacc("TRN2", target_bir_lowering=False)

    pts_d = nc.dram_tensor("pts", [C, N], f32r, kind="ExternalInput")
    pts2_d = nc.dram_tensor("pts2", [C, N], f32, kind="ExternalInput")
    w2t_d = nc.dram_tensor("w2t", [2, 128, 128], f32r, kind="ExternalInput")
    bias2_d = nc.dram_tensor("bias2", [2, 128], f32, kind="ExternalInput")
    ident_d = nc.dram_tensor("ident", [128, 128], f32r, kind="ExternalInput")
    identb_d = nc.dram_tensor("identb", [128, 128], bf16, kind="ExternalInput")
    indq_d = nc.dram_tensor("indq", [4, 128], f32r, kind="ExternalInput")
    indz_d = nc.dram_tensor("indz", [128, 4], f32r, kind="ExternalInput")
    indic_d = nc.dram_tensor("indic", [128, 8], f32r, kind="ExternalInput")
    indt_d = nc.dram_tensor("indt", [8, 128], f32r, kind="ExternalInput")
    gb_d = nc.dram_tensor("gb", [2, 128, 2], f32, kind="ExternalInput")
    out_d = nc.dram_tensor("out", [C, N], f32, kind="ExternalOutput")

    bf = {
        "t2p": int(os.environ.get("GSA_B_T2P", "2")),
        "vtp": int(os.environ.get("GSA_B_VTP", "2")),
        "elut": int(os.environ.get("GSA_B_ELUT", "2")),
        "ep": int(os.environ.get("GSA_B_EP", "2")),
        "misc": int(os.environ.get("GSA_B_MISC", "2")),
        "scps": int(os.environ.get("GSA_B_SCPS", "2")),
        "avps": int(os.environ.get("GSA_B_AVPS", "2")),
        "unroll": int(os.environ.get("GSA_UNROLL", "2")),
    }
    with tile.TileContext(nc) as tc:
        with tc.tile_pool(name="consts", bufs=1) as cp, \
             tc.tile_pool(name="t2p", bufs=bf["t2p"]) as t2p, \
             tc.tile_pool(name="vtp", bufs=bf["vtp"]) as vtp, \
             tc.tile_pool(name="elut", bufs=bf["elut"]) as elut, \
             tc.tile_pool(name="ep", bufs=bf["ep"]) as ep, \
             tc.tile_pool(name="misc", bufs=bf["misc"]) as misc, \
             tc.tile_pool(name="scps", bufs=bf["scps"], space="PSUM") as scps, \
             tc.tile_pool(name="avps", bufs=bf["avps"], space="PSUM") as avps, \
             tc.tile_pool(name="trbc", bufs=2, space="PSUM") as trbc:

            # ---- load constants / inputs ----
            pts_sb = cp.tile([128, N], f32r)
            nc.sync.dma_start(out=pts_sb, in_=pts_d[:])
            w2t_sb = cp.tile([128, 2, 128], f32r)
            nc.sync.dma_start(
                out=w2t_sb,
                in_=bass.AP(tensor=w2t_d, offset=0,
                            ap=[[128, 128], [128 * 128, 2], [1, 128]]))
            bias2_sb = cp.tile([128, 2], f32)
            nc.sync.dma_start(
                out=bias2_sb,
                in_=bass.AP(tensor=bias2_d, offset=0, ap=[[1, 128], [128, 2]]))
            ident_sb = cp.tile([128, 128], f32r)
            nc.sync.dma_start(out=ident_sb, in_=ident_d[:])
            identb_sb = cp.tile([128, 128], bf16)
            nc.sync.dma_start(out=identb_sb, in_=identb_d[:])
            indq_sb = cp.tile([4, 128], f32r)
            nc.sync.dma_start(out=indq_sb, in_=indq_d[:])
            indz_sb = cp.tile([128, 4], f32r)
            nc.sync.dma_start(out=indz_sb, in_=indz_d[:])
            indic_sb = cp.tile([128, 8], f32r)
            nc.sync.dma_start(out=indic_sb, in_=indic_d[:])
            indt_sb = cp.tile([8, 128], f32r)
            nc.sync.dma_start(out=indt_sb, in_=indt_d[:])
            gb_sb = cp.tile([128, 2, 2], f32)
            nc.sync.dma_start(
                out=gb_sb,
                in_=bass.AP(tensor=gb_d, offset=0,
                            ap=[[2, 128], [256, 2], [1, 2]]))
            # fast-rsqrt magic constant (int32) for the GroupNorm rstd
            i32 = mybir.dt.int32
            magic_sb = cp.tile([8, 1], i32)
            nc.vector.memset(magic_sb, 0x5F3759DF)
            # shuffled-channel residual input: partition 32j+c of quad q holds
            # points channel c*8+(4q+j)
            ptsq_sb = cp.tile([128, 2, N], f32)
            nc.vector.memset(ptsq_sb, 0.0)
            for q in range(2):
                for j in range(4):
                    nc.sync.dma_start(
                        out=ptsq_sb[32 * j:32 * j + 16, q, :],
                        in_=bass.AP(tensor=pts2_d, offset=(4 * q + j) * N,
                                    ap=[[8 * N, 16], [1, N]]))

            def rep_body():
                # ---- conv (grouped 1x1) into T2 layout + bias ----
                t2_sb = t2p.tile([128, 2, N], bf16, tag="t2")
                for q in range(2):
                    for h2 in range(2):
                        cps = scps.tile([128, 512], f32, tag="sc")
                        nc.tensor.matmul(
                            out=cps[:],
                            lhsT=w2t_sb[:, q, :],
                            rhs=pts_sb[:, h2 * 512:(h2 + 1) * 512],
                            start=True, stop=True)
                        nc.vector.tensor_scalar(
                            out=t2_sb[:, q, h2 * 512:(h2 + 1) * 512], in0=cps[:],
                            scalar1=bias2_sb[:, q:q + 1],
                            scalar2=None, op0=ALU.add)

                # ---- transposes (for V^T) + elu ----
                # vT layout per quad: [128 n-sub, k-tile, strip j, 32] where cols
                # 0..15 = elu(t), col 16 = ones (for Z row)
                # elu reads min/max straight from the transpose PSUM (no staging
                # copy); 4 transposes share one PSUM tile per half.
                vt_sb = vtp.tile([128, 2, 8, 4, 32], bf16, tag="vt")
                for q in range(2):
                    mn = elut.tile([128, N], f32, tag="e1")
                    mx = elut.tile([128, N], f32, tag="e2")
                    for g4 in range(2):
                        trp = trbc.tile([128, 512], bf16, tag="tb")
                        for kk in range(4):
                            k = 4 * g4 + kk
                            nc.tensor.transpose(
                                out=trp[:, kk * 128:(kk + 1) * 128],
                                in_=t2_sb[:, q, k * 128:(k + 1) * 128],
                                identity=identb_sb[:])
                        nc.vector.tensor_scalar_min(
                            mn[:, g4 * 512:(g4 + 1) * 512], trp[:], 0.0)
                        nc.vector.tensor_scalar_max(
                            mx[:, g4 * 512:(g4 + 1) * 512], trp[:], 0.0)
                    ex = elut.tile([128, N], f32, tag="e3")
                    nc.scalar.activation(out=ex[:], in_=mn[:], func=AF.Exp)
                    nc.gpsimd.tensor_add(
                        out=vt_sb[:, q, :, :, :].rearrange("p k j c -> p (k j c)"),
                        in0=mx[:], in1=ex[:])
                nc.gpsimd.memset(vt_sb[:, :, :, :, 16:17], 1.0)

                # ---- scores + exp + attn@V, software-pipelined ----
                # Block p emits scores(p+1) chunks interleaved with attnV(p)
                # matmuls so the PE queue never drains while Act runs the
                # exps for the NEXT pair: without this the in-order PE queue
                # stalls on exp(p) before it can start scores(p+1).
                av_tiles = {}
                et_tiles = {}
                y_sb = t2p.tile([128, 2, N], f32, tag="y")
                sums = misc.tile([128, 4], f32r, tag="sums")

                def emit_scores_chunk(p, i, h):
                    q, lp = p // 2, p % 2
                    et = et_tiles[p]
                    if ablate & 16:  # skip scores+exp entirely
                        return
                    sps = scps.tile([128, N], f32, tag="sc")
                    if True:
                        for rt in range(2):
                            j = 2 * lp + rt
                            nc.tensor.matmul(
                                out=sps[:, rt * 512:(rt + 1) * 512],
                                lhsT=t2_sb[32 * j:32 * j + 32, q,
                                           i * 128:(i + 1) * 128],
                                rhs=t2_sb[32 * j:32 * j + 32, q,
                                          h * 512:(h + 1) * 512],
                                start=True, stop=True,
                                tile_position=(32 * j, 0))
                    if not ablate & 8:
                        nc.scalar.activation(
                            out=et[:, i, h, :, :],
                            in_=sps[:].rearrange("p (r n) -> p r n", r=2),
                            func=AF.Exp, scale=0.25)

                # upper-right superblock (row blocks 0..3 x col blocks 4..7)
                # is the transpose of the computed lower-left; materialize it
                # with PE transposes of the exp'd bf16 tiles + one DVE 2x
                # copy per pair of row blocks, skipping 4 exp chunks per pair.
                def gen_mirror(p):
                    q, lp = p // 2, p % 2
                    et = et_tiles[p]
                    for rt in range(2):
                        for ip in range(0, 4, 2):  # dest row-block pair
                            trp2 = trbc.tile([128, 2, 4, 128], bf16, tag="tb")
                            for ipo in range(2):
                                for bb in range(4):  # col block B-4
                                    yield lambda rt=rt, ip=ip, ipo=ipo, bb=bb, \
                                        trp2=trp2, et=et: \
                                        nc.tensor.transpose(
                                            out=trp2[:, ipo, bb, :],
                                            in_=et[:, bb + 4, 0, rt,
                                                   (ip + ipo) * 128:
                                                   (ip + ipo + 1) * 128],
                                            identity=identb_sb[:])
                            yield lambda rt=rt, ip=ip, trp2=trp2, et=et: \
                                nc.vector.tensor_copy(
                                    out=et[:, ip:ip + 2, 1, rt, :],
                                    in_=trp2[:])

                def gen_attnv(p):
                    q, lp = p // 2, p % 2
                    et = et_tiles[p]
                    for h in range(2):
                        if lp == 0:
                            av_new = avps.tile([128, 512], f32, tag="av")
                            av_tiles[(q, h)] = av_new
                        av = av_tiles[(q, h)]
                        for rt in range(2):
                            j = 2 * lp + rt
                            for k in range(8):
                                # full 32-wide strip: vt cols 16:31 are 1.0
                                # (elu(0)); start flag zero-fills rows 17:31
                                # so no PSUM memset is needed before reads.
                                yield lambda av=av, j=j, k=k, h=h, rt=rt, q=q: \
                                    nc.tensor.matmul(
                                        out=av[32 * j:32 * j + 32, :],
                                        lhsT=vt_sb[:, q, k, j, 0:32],
                                        rhs=et_tiles[p][:, k, h, rt, :],
                                        start=(k == 0), stop=(k == 7),
                                        tile_position=(0, 32 * j))

                def normalize(q):
                    if True:
                        # gather the 4 Z rows (psum rows 32j+16) with an
                        # indicator matmul instead of 16 tiny DMAs
                        rz = misc.tile([4, N], f32r, tag="rz")
                        for h in range(2):
                            av = av_tiles[(q, h)]
                            stg = misc.tile([128, 512], f32r, tag="stg")
                            nc.vector.tensor_copy(out=stg[:], in_=av[:])
                            zp = trbc.tile([4, 512], f32, tag="tb")
                            nc.tensor.matmul(
                                out=zp[:], lhsT=indz_sb[:], rhs=stg[:],
                                start=True, stop=True)
                            with nc.allow_low_precision(reason="1/Z at f32r"):
                                nc.vector.reciprocal(
                                    rz[:, h * 512:(h + 1) * 512], zp[:])
                        for h in range(2):
                            bcp = trbc.tile([128, 512], f32, tag="tb")
                            nc.tensor.matmul(
                                out=bcp[:], lhsT=indq_sb[:],
                                rhs=rz[:, h * 512:(h + 1) * 512],
                                start=True, stop=True)
                            bcs = misc.tile([128, 512], f32, tag="bcs")
                            nc.vector.tensor_copy(out=bcs[:], in_=bcp[:])
                            nc.vector.tensor_mul(
                                out=y_sb[:, q, h * 512:(h + 1) * 512],
                                in0=av_tiles[(q, h)][:], in1=bcs[:])
                        nc.vector.tensor_add(out=y_sb[:, q, :], in0=y_sb[:, q, :],
                                             in1=ptsq_sb[:, q, :])
                        with nc.allow_low_precision(reason="GN sums at f32r"):
                            nc.vector.tensor_reduce(
                                out=sums[:, 2 * q:2 * q + 1], in_=y_sb[:, q, :],
                                axis=mybir.AxisListType.X, op=ALU.add)
                        sq = elut.tile([128, N], f32, tag="sq")
                        nc.gpsimd.tensor_mul(sq[:], y_sb[:, q, :], y_sb[:, q, :])
                        with nc.allow_low_precision(reason="GN sums at f32r"):
                            nc.vector.tensor_reduce(
                                out=sums[:, 2 * q + 1:2 * q + 2], in_=sq[:],
                                axis=mybir.AxisListType.X, op=ALU.add)

                # driver: prologue scores(0), then block p = scores(p+1)
                # chunks interleaved with mirror(p) transposes/copies and
                # attnV(p) matmuls (mirror first so the upper et tiles are
                # in SBUF before the h=1 attnV matmuls consume them).
                CHUNKS = [(i, h) for i in range(8) for h in range(2)
                          if not (h == 1 and i < 4)]
                et0 = ep.tile([128, 8, 2, 2, 512], bf16, tag="E")
                et_tiles[0] = et0
                for i, h in CHUNKS:
                    emit_scores_chunk(0, i, h)
                for p in range(4):
                    mir = [] if ablate & 64 else list(gen_mirror(p))
                    gen = iter(()) if ablate & 32 else gen_attnv(p)
                    if ablate & 32:
                        for h in range(2):
                            if p % 2 == 0:
                                av_stub = avps.tile([128, 512], f32, tag="av")
                                nc.vector.memset(av_stub, 1.0)
                                av_tiles[(p // 2, h)] = av_stub
                    if p < 3:
                        etn = ep.tile([128, 8, 2, 2, 512], bf16, tag="E")
                        et_tiles[p + 1] = etn
                        for ci, (i, h) in enumerate(CHUNKS):
                            emit_scores_chunk(p + 1, i, h)
                            # mirror ops early (9 per chunk over chunks 0..3)
                            if ci < 4:
                                for _ in range(9):
                                    if mir:
                                        mir.pop(0)()
                            for _ in range(3):
                                nxt = next(gen, None)
                                if nxt is not None:
                                    nxt()
                    for op in mir:
                        op()
                    for thunk in gen:
                        thunk()
                    if p % 2 == 1:
                        normalize(p // 2)

                # ---- GroupNorm (combine both quads; groups span q=0 and q=1) ----
                stp = trbc.tile([8, 4], f32, tag="tb")
                nc.tensor.matmul(out=stp[:], lhsT=indic_sb[:],
                                 rhs=sums[:], start=True, stop=True)
                st = misc.tile([8, 4], f32, tag="st")
                nc.vector.tensor_copy(out=st[:], in_=stp[:])
                # mean/var per group (16 ch x 1024 = 16384 elements)
                mv = misc.tile([8, 4], f32, tag="mv")  # cols: mean, var+eps, rstd, -
                nc.vector.tensor_add(out=mv[:, 0:1], in0=st[:, 0:1], in1=st[:, 2:3])
                nc.vector.tensor_scalar_mul(mv[:, 0:1], mv[:, 0:1], 1.0 / 16384.0)
                nc.vector.tensor_add(out=mv[:, 1:2], in0=st[:, 1:2], in1=st[:, 3:4])
                nc.vector.tensor_scalar_mul(mv[:, 1:2], mv[:, 1:2], 1.0 / 16384.0)
                msq = misc.tile([8, 1], f32, tag="msq")
                nc.vector.tensor_mul(msq[:], mv[:, 0:1], mv[:, 0:1])
                nc.vector.tensor_sub(out=mv[:, 1:2], in0=mv[:, 1:2], in1=msq[:])
                nc.vector.tensor_scalar_add(mv[:, 1:2], mv[:, 1:2], EPS)
                # rstd = (var+eps)^-0.5 via fast-inverse-sqrt on DVE: bit-trick
                # seed + 2 Newton steps (rel err ~1e-8); avoids the Ln/Exp
                # act-table churn with the Exp-heavy attention loop.
                sh = misc.tile([8, 1], i32, tag="sh")
                nc.vector.tensor_scalar(
                    out=sh[:], in0=mv[:, 1:2].bitcast(i32), scalar1=1,
                    scalar2=None, op0=ALU.arith_shift_right)
                yb = misc.tile([8, 1], i32, tag="yb")
                nc.vector.tensor_sub(out=yb[:], in0=magic_sb[:], in1=sh[:])
                yf = yb[:].bitcast(f32)
                for _ in range(2):
                    nc.vector.tensor_mul(msq[:], yf, yf)
                    nc.vector.tensor_mul(msq[:], msq[:], mv[:, 1:2])
                    nc.vector.tensor_scalar(
                        out=msq[:], in0=msq[:], scalar1=-0.5, scalar2=1.5,
                        op0=ALU.mult, op1=ALU.add)
                    nc.vector.tensor_mul(yf, yf, msq[:])
                nc.vector.tensor_copy(out=mv[:, 2:3], in_=yf)
                grp = misc.tile([8, 2], f32r, tag="grp")  # (rstd, mean)
                nc.vector.tensor_copy(out=grp[:, 0:1], in_=mv[:, 2:3])
                nc.vector.tensor_copy(out=grp[:, 1:2], in_=mv[:, 0:1])
                bkp = trbc.tile([128, 2], f32, tag="tb")
                nc.tensor.matmul(out=bkp[:], lhsT=indt_sb[:],
                                 rhs=grp[:], start=True, stop=True)
                pp = misc.tile([128, 2], f32, tag="pp")  # per-partition rstd, mean
                nc.vector.tensor_copy(out=pp[:], in_=bkp[:])
                for q in range(2):
                    sc = misc.tile([128, 1], f32, tag="scq")
                    bi = misc.tile([128, 1], f32, tag="biq")
                    nc.vector.tensor_mul(sc[:], gb_sb[:, q, 0:1], pp[:, 0:1])
                    nc.vector.tensor_mul(bi[:], pp[:, 1:2], sc[:])
                    nc.vector.tensor_sub(out=bi[:], in0=gb_sb[:, q, 1:2], in1=bi[:])
                    eng = nc.gpsimd if q == 0 else nc.vector
                    eng.tensor_scalar(
                        out=y_sb[:, q, :], in0=y_sb[:, q, :], scalar1=sc[:],
                        scalar2=bi[:], op0=ALU.mult, op1=ALU.add)
                    for j in range(4):
                        nc.sync.dma_start(
                            out=bass.AP(tensor=out_d, offset=(4 * q + j) * N,
                                        ap=[[8 * N, 16], [1, N]]),
                            in_=y_sb[32 * j:32 * j + 16, q, :])

            if ablate & 4:  # straight-line unroll (sim analysis only)
                for _ in range(reps):
                    rep_body()
            else:
                u = bf["unroll"]
                n_loop, rem = divmod(reps, u)
                if n_loop >= 1:
                    with tc.For_i(0, n_loop, staggered_reset=True):
                        for _ in range(u):
                            rep_body()
                for _ in range(rem):
                    rep_body()

    nc.compile()
    return nc


def _make_runner(nc):
    """Trace/lower/compile the SPMD executable ONCE and return a fast runner.

    run_bass_kernel_spmd re-creates a fresh jax.jit on every call, so each
    kernel() invocation pays full retrace + XLA/NEFF compile (~0.5s+, scaling
    with program length). Caching the jitted callable makes repeat calls pure
    dispatch + device execution.
    """
    import jax
    from jax.experimental.shard_map import shard_map
    from jax.sharding import Mesh, PartitionSpec

    from concourse.bass2jax import (
        _bass_exec_p,
        install_neuronx_cc_hook,
        partition_id_tensor,
    )

    install_neuronx_cc_hook()
    partition_name = (nc.partition_id_tensor.name
                      if nc.partition_id_tensor else None)
    in_names = []
    out_names = []
    out_avals = []
    zero_shapes = []
    for alloc in nc.m.functions[0].allocations:
        if not isinstance(alloc, mybir.MemoryLocationSet):
            continue
        name = alloc.memorylocations[0].name
        if alloc.kind == "ExternalInput":
            if name != partition_name:
                in_names.append(name)
        elif alloc.kind == "ExternalOutput":
            shape = tuple(alloc.tensor_shape)
            dtype = mybir.dt.np(alloc.dtype)
            out_names.append(name)
            out_avals.append(jax.core.ShapedArray(shape, dtype))
            zero_shapes.append((shape, dtype))
    n_params = len(in_names)
    n_outs = len(out_avals)
    bind_in_names = list(in_names) + list(out_names)
    if partition_name is not None:
        bind_in_names.append(partition_name)
    donate = tuple(range(n_params, n_params + n_outs))

    def _body(*args):
        operands = list(args)
        if partition_name is not None:
            operands.append(partition_id_tensor())
        outs = _bass_exec_p.bind(
            *operands,
            out_avals=tuple(out_avals),
            in_names=tuple(bind_in_names),
            out_names=tuple(out_names),
            lowering_input_output_aliases=(),
            sim_require_finite=True,
            sim_require_nnan=True,
            nc=nc,
        )
        return tuple(outs)

    import hashlib

    import jax.numpy as jnp
    from jax.sharding import NamedSharding

    devices = jax.devices()[:B]
    mesh = Mesh(np.asarray(devices), ("core",))
    in_specs = (PartitionSpec("core"),) * (n_params + n_outs)
    out_specs = (PartitionSpec("core"),) * n_outs
    jitted = jax.jit(
        shard_map(_body, mesh=mesh, in_specs=in_specs, out_specs=out_specs,
                  check_rep=False),
        donate_argnums=donate, keep_unused=True)

    sharding = NamedSharding(mesh, PartitionSpec("core"))

    def _zeros():
        return tuple(jnp.zeros((B * s[0], *s[1:]), d) for (s, d) in zero_shapes)

    zeros_fn = jax.jit(_zeros, out_shardings=(sharding,) * n_outs)
    dev_cache = {}

    def run(in_maps):
        import time as _time
        prof = bool(int(os.environ.get("GSA_PROF", "0")))
        t0 = _time.time()
        per_core = [[np.asarray(m[name]) for name in in_names]
                    for m in in_maps]
        concat_in = [
            np.concatenate([per_core[c][i] for c in range(B)], axis=0)
            for i in range(n_params)
        ]
        h = hashlib.blake2b()
        for a in concat_in:
            h.update(a.tobytes())
        key = h.digest()
        t1 = _time.time()
        if key in dev_cache:
            dev_in = dev_cache[key]
        else:
            dev_in = [jax.device_put(a, sharding) for a in concat_in]
            jax.block_until_ready(dev_in)
            dev_cache.clear()
            dev_cache[key] = dev_in
        t2 = _time.time()
        zf = zeros_fn()
        jax.block_until_ready(zf)
        t2b = _time.time()
        out_arrs = jitted(*dev_in, *zf)
        jax.block_until_ready(out_arrs)
        t3 = _time.time()
        _CACHE["exec_wall"] = t3 - t2b
        res = [
            {name: np.asarray(out_arrs[i]).reshape(B, *out_avals[i].shape)[c]
             for i, name in enumerate(out_names)}
            for c in range(B)
        ]
        t4 = _time.time()
        if prof:
            print(f"  [prof] concat+hash {t1-t0:.4f}s  upload {t2-t1:.4f}s  "
                  f"exec {t3-t2:.4f}s  download {t4-t3:.4f}s")
        return res

    return run


def _host_consts(w, b, gamma, beta):
    W = w.reshape(C, CG).astype(np.float32)
    w2t = np.zeros((2, 128, 128), np.float32)
    bias2 = np.zeros((2, 128), np.float32)
    gb = np.zeros((2, 128, 2), np.float32)
    for q in range(2):
        for j in range(4):
            g = 4 * q + j
            for o in range(CG):
                w2t[q, g * CG:(g + 1) * CG, 32 * j + o] = W[g * CG + o]
                bias2[q, 32 * j + o] = b[g * CG + o]
            for c in range(CG):
                gb[q, 32 * j + c, 0] = gamma[c * 8 + g]
                gb[q, 32 * j + c, 1] = beta[c * 8 + g]
    import ml_dtypes
    ident = np.eye(128, dtype=np.float32)
    identb = np.eye(128, dtype=ml_dtypes.bfloat16)
    indq = np.zeros((4, 128), np.float32)
    for j in range(4):
        indq[j, 32 * j:32 * (j + 1)] = 1.0
    indz = np.zeros((128, 4), np.float32)
    for j in range(4):
        indz[32 * j + 16, j] = 1.0
    indic = np.zeros((128, 8), np.float32)
    indt = np.zeros((8, 128), np.float32)
    for p in range(128):
        c = p % 32
        if c < 16:
            indic[p, c // 2] = 1.0
            indt[c // 2, p] = 1.0
    return dict(w2t=w2t, bias2=bias2, ident=ident, identb=identb, indq=indq,
                indz=indz, indic=indic, indt=indt, gb=gb)


def kernel(points, w, b, gamma, beta):
    points = np.ascontiguousarray(np.asarray(points, np.float32))
    consts = _host_consts(np.asarray(w, np.float32), np.asarray(b, np.float32),
                          np.asarray(gamma, np.float32),
                          np.asarray(beta, np.float32))
    reps = int(os.environ.get("GSA_REPS", "1"))
    ablate = int(os.environ.get("GSA_ABLATE", "0"))
    in_maps = [dict(pts=points[core], pts2=points[core], **consts) for core in range(B)]
    trace = bool(int(os.environ.get("GSA_TRACE", "0")))
    if trace:
        if ("nc", reps, ablate) not in _CACHE:
            _CACHE[("nc", reps, ablate)] = _build_nc(reps, ablate)
        nc = _CACHE[("nc", reps, ablate)]
        res = run_bass_kernel_spmd(nc, in_maps, core_ids=list(range(B)),
                                   trace=trace)
        _CACHE["exec_time_ns"] = res.exec_time_ns
        _CACHE["results"] = res
        return np.stack([res.results[core]["out"] for core in range(B)], axis=0)
    key = ("runner", reps, ablate)
    if key not in _CACHE:
        if ("nc", reps, ablate) not in _CACHE:
            _CACHE[("nc", reps, ablate)] = _build_nc(reps, ablate)
        _CACHE[key] = _make_runner(_CACHE[("nc", reps, ablate)])
    results = _CACHE[key](in_maps)
    return np.stack([results[core]["out"] for core in range(B)], axis=0)



# revision 53
# speedup vs baseline: 1.6368x; 1.0676x over previous
"""GroupShuffleAttention Trainium2 kernel.

Per-core = one batch (B=8 over 8 cores). Layout tricks:
- grouped 1x1 conv as one dense matmul with a block-structured, 32-row-spaced
  weight ("T2" layout: quad q holds groups 4q+j at partition rows 32j..32j+15).
- per-group NxN scores via row-tiled (tile_position) f32r matmuls; the score
  matrix is symmetric so the same stored exp(scores) tiles serve as both
  softmax rows and the attn@V contraction operand -- no NxN transposes.
- softmax denominator Z comes free as a 17th "ones" column in the attn@V
  stationary operand (col-tiled matmuls place each group at psum rows 32j).
- elu(t) = relu(t) + exp(min(t,0)) - 1; the -1 contributes exactly -1 to the
  attention output (att columns sum to 1) and then cancels in GroupNorm
  (shift-invariant), so it is dropped entirely.
- GroupNorm stats and per-partition broadcast via tiny indicator matmuls.
"""

import os
import sys

sys.path.insert(0, "/opt/trn_rl_repo")

import numpy as np

import concourse.bass as bass
import concourse.mybir as mybir
import concourse.tile as tile
from concourse import bacc
from concourse.bass_utils import run_bass_kernel_spmd

f32 = mybir.dt.float32
f32r = mybir.dt.float32r
bf16 = mybir.dt.bfloat16
AF = mybir.ActivationFunctionType
ALU = mybir.AluOpType

B, C, N, G = 8, 128, 1024, 8
CG = 16
EPS = 1e-5

_CACHE = {}


def _build_nc(reps=1, ablate=0):
    nc = bacc.Bacc("TRN2", target_bir_lowering=False)

    pts_d = nc.dram_tensor("pts", [C, N], f32r, kind="ExternalInput")
    pts2_d = nc.dram_tensor("pts2", [C, N], f32, kind="ExternalInput")
    w2t_d = nc.dram_tensor("w2t", [2, 128, 128], f32r, kind="ExternalInput")
    bias2_d = nc.dram_tensor("bias2", [2, 128], f32, kind="ExternalInput")
    ident_d = nc.dram_tensor("ident", [128, 128], f32r, kind="ExternalInput")
    identb_d = nc.dram_tensor("identb", [128, 128], bf16, kind="ExternalInput")
    indq_d = nc.dram_tensor("indq", [4, 128], f32r, kind="ExternalInput")
    indz_d = nc.dram_tensor("indz", [128, 4], f32r, kind="ExternalInput")
    indz2_d = nc.dram_tensor("indz2", [128, 128], f32r, kind="ExternalInput")
    indic_d = nc.dram_tensor("indic", [128, 8], f32r, kind="ExternalInput")
    indt_d = nc.dram_tensor("indt", [8, 128], f32r, kind="ExternalInput")
    gb_d = nc.dram_tensor("gb", [2, 128, 2], f32, kind="ExternalInput")
    out_d = nc.dram_tensor("out", [C, N], f32, kind="ExternalOutput")

    bf = {
        "t2p": int(os.environ.get("GSA_B_T2P", "2")),
        "vtp": int(os.environ.get("GSA_B_VTP", "2")),
        "elut": int(os.environ.get("GSA_B_ELUT", "2")),
        "ep": int(os.environ.get("GSA_B_EP", "2")),
        "misc": int(os.environ.get("GSA_B_MISC", "2")),
        "scps": int(os.environ.get("GSA_B_SCPS", "2")),
        "avps": int(os.environ.get("GSA_B_AVPS", "2")),
        "unroll": int(os.environ.get("GSA_UNROLL", "2")),
    }
    with tile.TileContext(nc) as tc:
        with tc.tile_pool(name="consts", bufs=1) as cp, \
             tc.tile_pool(name="t2p", bufs=bf["t2p"]) as t2p, \
             tc.tile_pool(name="vtp", bufs=bf["vtp"]) as vtp, \
             tc.tile_pool(name="elut", bufs=bf["elut"]) as elut, \
             tc.tile_pool(name="ep", bufs=bf["ep"]) as ep, \
             tc.tile_pool(name="misc", bufs=bf["misc"]) as misc, \
             tc.tile_pool(name="scps", bufs=bf["scps"], space="PSUM") as scps, \
             tc.tile_pool(name="avps", bufs=bf["avps"], space="PSUM") as avps, \
             tc.tile_pool(name="trbc", bufs=2, space="PSUM") as trbc:

            # ---- load constants / inputs ----
            pts_sb = cp.tile([128, N], f32r)
            nc.sync.dma_start(out=pts_sb, in_=pts_d[:])
            w2t_sb = cp.tile([128, 2, 128], f32r)
            nc.sync.dma_start(
                out=w2t_sb,
                in_=bass.AP(tensor=w2t_d, offset=0,
                            ap=[[128, 128], [128 * 128, 2], [1, 128]]))
            bias2_sb = cp.tile([128, 2], f32)
            nc.sync.dma_start(
                out=bias2_sb,
                in_=bass.AP(tensor=bias2_d, offset=0, ap=[[1, 128], [128, 2]]))
            ident_sb = cp.tile([128, 128], f32r)
            nc.sync.dma_start(out=ident_sb, in_=ident_d[:])
            identb_sb = cp.tile([128, 128], bf16)
            nc.sync.dma_start(out=identb_sb, in_=identb_d[:])
            indq_sb = cp.tile([4, 128], f32r)
            nc.sync.dma_start(out=indq_sb, in_=indq_d[:])
            indz_sb = cp.tile([128, 4], f32r)
            nc.sync.dma_start(out=indz_sb, in_=indz_d[:])
            indz2_sb = cp.tile([128, 128], f32r)
            nc.sync.dma_start(out=indz2_sb, in_=indz2_d[:])
            indic_sb = cp.tile([128, 8], f32r)
            nc.sync.dma_start(out=indic_sb, in_=indic_d[:])
            indt_sb = cp.tile([8, 128], f32r)
            nc.sync.dma_start(out=indt_sb, in_=indt_d[:])
            gb_sb = cp.tile([128, 2, 2], f32)
            nc.sync.dma_start(
                out=gb_sb,
                in_=bass.AP(tensor=gb_d, offset=0,
                            ap=[[2, 128], [256, 2], [1, 2]]))
            # fast-rsqrt magic constant (int32) for the GroupNorm rstd
            i32 = mybir.dt.int32
            magic_sb = cp.tile([8, 1], i32)
            nc.vector.memset(magic_sb, 0x5F3759DF)
            # shuffled-channel residual input: partition 32j+c of quad q holds
            # points channel c*8+(4q+j)
            ptsq_sb = cp.tile([128, 2, N], f32)
            nc.vector.memset(ptsq_sb, 0.0)
            for q in range(2):
                for j in range(4):
                    nc.sync.dma_start(
                        out=ptsq_sb[32 * j:32 * j + 16, q, :],
                        in_=bass.AP(tensor=pts2_d, offset=(4 * q + j) * N,
                                    ap=[[8 * N, 16], [1, N]]))

            def rep_body():
                # ---- conv (grouped 1x1) into T2 layout + bias ----
                t2_sb = t2p.tile([128, 2, N], bf16, tag="t2")
                for q in range(2):
                    for h2 in range(2):
                        cps = scps.tile([128, 512], f32, tag="sc")
                        nc.tensor.matmul(
                            out=cps[:],
                            lhsT=w2t_sb[:, q, :],
                            rhs=pts_sb[:, h2 * 512:(h2 + 1) * 512],
                            start=True, stop=True)
                        nc.vector.tensor_scalar(
                            out=t2_sb[:, q, h2 * 512:(h2 + 1) * 512], in0=cps[:],
                            scalar1=bias2_sb[:, q:q + 1],
                            scalar2=None, op0=ALU.add)

                # ---- transposes (for V^T) + elu ----
                # vT layout per quad: [128 n-sub, k-tile, strip j, 32] where cols
                # 0..15 = elu(t), col 16 = ones (for Z row)
                # elu reads min/max straight from the transpose PSUM (no staging
                # copy); 4 transposes share one PSUM tile per half.
                vt_sb = vtp.tile([128, 2, 8, 4, 32], bf16, tag="vt")
                for q in range(2):
                    mn = elut.tile([128, N], bf16, tag="e1")
                    mx = elut.tile([128, N], bf16, tag="e2")
                    for g4 in range(2):
                        trp = trbc.tile([128, 512], bf16, tag="tb")
                        for kk in range(4):
                            k = 4 * g4 + kk
                            nc.tensor.transpose(
                                out=trp[:, kk * 128:(kk + 1) * 128],
                                in_=t2_sb[:, q, k * 128:(k + 1) * 128],
                                identity=identb_sb[:])
                        nc.vector.tensor_scalar_min(
                            mn[:, g4 * 512:(g4 + 1) * 512], trp[:], 0.0)
                        nc.vector.tensor_scalar_max(
                            mx[:, g4 * 512:(g4 + 1) * 512], trp[:], 0.0)
                    ex = elut.tile([128, N], bf16, tag="e3")
                    nc.scalar.activation(out=ex[:], in_=mn[:], func=AF.Exp)
                    nc.gpsimd.tensor_add(
                        out=vt_sb[:, q, :, :, :].rearrange("p k j c -> p (k j c)"),
                        in0=mx[:], in1=ex[:])
                nc.gpsimd.memset(vt_sb[:, :, :, :, 16:17], 1.0)

                # ---- scores + exp + attn@V, software-pipelined ----
                # Block p emits scores(p+1) chunks interleaved with attnV(p)
                # matmuls so the PE queue never drains while Act runs the
                # exps for the NEXT pair: without this the in-order PE queue
                # stalls on exp(p) before it can start scores(p+1).
                av_tiles = {}
                et_tiles = {}
                y_sb = t2p.tile([128, 2, N], f32, tag="y")
                sums = misc.tile([128, 8], f32r, tag="sums")

                def emit_scores_chunk(p, i, h):
                    q, lp = p // 2, p % 2
                    et = et_tiles[p]
                    if ablate & 16:  # skip scores+exp entirely
                        return
                    sps = scps.tile([128, N], f32, tag="sc")
                    if True:
                        for rt in range(2):
                            j = 2 * lp + rt
                            nc.tensor.matmul(
                                out=sps[:, rt * 512:(rt + 1) * 512],
                                lhsT=t2_sb[32 * j:32 * j + 32, q,
                                           i * 128:(i + 1) * 128],
                                rhs=t2_sb[32 * j:32 * j + 32, q,
                                          h * 512:(h + 1) * 512],
                                start=True, stop=True,
                                tile_position=(32 * j, 0))
                    if not ablate & 8:
                        nc.scalar.activation(
                            out=et[:, i, h, :, :],
                            in_=sps[:].rearrange("p (r n) -> p r n", r=2),
                            func=AF.Exp, scale=0.25)

                # upper-right superblock (row blocks 0..3 x col blocks 4..7)
                # is the transpose of the computed lower-left; materialize it
                # with PE transposes of the exp'd bf16 tiles + one DVE 2x
                # copy per pair of row blocks, skipping 4 exp chunks per pair.
                def gen_mirror(p):
                    q, lp = p // 2, p % 2
                    et = et_tiles[p]
                    for rt in range(2):
                        for ip in range(0, 4, 2):  # dest row-block pair
                            trp2 = trbc.tile([128, 2, 4, 128], bf16, tag="tb")
                            for ipo in range(2):
                                for bb in range(4):  # col block B-4
                                    yield lambda rt=rt, ip=ip, ipo=ipo, bb=bb, \
                                        trp2=trp2, et=et: \
                                        nc.tensor.transpose(
                                            out=trp2[:, ipo, bb, :],
                                            in_=et[:, bb + 4, 0, rt,
                                                   (ip + ipo) * 128:
                                                   (ip + ipo + 1) * 128],
                                            identity=identb_sb[:])
                            yield lambda rt=rt, ip=ip, trp2=trp2, et=et: \
                                nc.vector.tensor_copy(
                                    out=et[:, ip:ip + 2, 1, rt, :],
                                    in_=trp2[:])

                def gen_attnv(p):
                    q, lp = p // 2, p % 2
                    et = et_tiles[p]
                    for h in range(2):
                        if lp == 0:
                            av_new = avps.tile([128, 512], f32, tag="av")
                            av_tiles[(q, h)] = av_new
                        av = av_tiles[(q, h)]
                        for rt in range(2):
                            j = 2 * lp + rt
                            for k in range(8):
                                # full 32-wide strip: vt cols 16:31 are 1.0
                                # (elu(0)); start flag zero-fills rows 17:31
                                # so no PSUM memset is needed before reads.
                                yield lambda av=av, j=j, k=k, h=h, rt=rt, q=q: \
                                    nc.tensor.matmul(
                                        out=av[32 * j:32 * j + 32, :],
                                        lhsT=vt_sb[:, q, k, j, 0:32],
                                        rhs=et_tiles[p][:, k, h, rt, :],
                                        start=(k == 0), stop=(k == 7),
                                        tile_position=(0, 32 * j))

                def normalize(q):
                    # reciprocal of the whole av tile (DVE cost is free-dim
                    # only, so 128 partitions cost the same as 4) -- then one
                    # indicator matmul broadcasts each strip's 1/Z row
                    # (partition 32j+16) back over the strip's 32 rows.
                    for h in range(2):
                        av = av_tiles[(q, h)]
                        rca = misc.tile([128, 512], f32r, tag="rca")
                        with nc.allow_low_precision(reason="1/Z at f32r"):
                            nc.vector.reciprocal(rca[:], av[:])
                        bcp = trbc.tile([128, 512], f32, tag="tb")
                        nc.tensor.matmul(
                            out=bcp[:], lhsT=indz2_sb[:], rhs=rca[:],
                            start=True, stop=True)
                        bcs = misc.tile([128, 512], f32, tag="bcs")
                        nc.scalar.activation(out=bcs[:], in_=bcp[:],
                                             func=AF.Copy)
                        yh = y_sb[:, q, h * 512:(h + 1) * 512]
                        nc.vector.tensor_mul(out=yh, in0=av[:], in1=bcs[:])
                        # residual add fused with the GN sum accumulator;
                        # square fused with the GN sum-of-squares accumulator
                        c = 4 * q + 2 * h
                        nofuse = int(os.environ.get("GSA_NOFUSE", "0"))
                        if nofuse in (0, 2):  # fused residual+sum
                            with nc.allow_low_precision(reason="GN@f32r"):
                                nc.vector.scalar_tensor_tensor(
                                    out=yh, in0=yh, scalar=1.0,
                                    in1=ptsq_sb[:, q, h * 512:(h + 1) * 512],
                                    op0=ALU.mult, op1=ALU.add,
                                    accum_out=sums[:, c:c + 1])
                        else:
                            nc.vector.tensor_add(
                                out=yh, in0=yh,
                                in1=ptsq_sb[:, q, h * 512:(h + 1) * 512])
                            with nc.allow_low_precision(reason="GN@f32r"):
                                nc.vector.tensor_reduce(
                                    out=sums[:, c:c + 1], in_=yh,
                                    axis=mybir.AxisListType.X, op=ALU.add)
                        if nofuse in (0, 3):  # fused square+sum
                            sq = elut.tile([128, 512], f32, tag="sq")
                            with nc.allow_low_precision(reason="GN@f32r"):
                                nc.vector.scalar_tensor_tensor(
                                    out=sq[:], in0=yh, scalar=1.0,
                                    in1=yh, op0=ALU.mult, op1=ALU.mult,
                                    accum_out=sums[:, c + 1:c + 2])
                        else:
                            sq = elut.tile([128, 512], f32, tag="sq")
                            nc.gpsimd.tensor_mul(sq[:], yh, yh)
                            with nc.allow_low_precision(reason="GN@f32r"):
                                nc.vector.tensor_reduce(
                                    out=sums[:, c + 1:c + 2], in_=sq[:],
                                    axis=mybir.AxisListType.X, op=ALU.add)

                # driver: prologue scores(0), then block p = scores(p+1)
                # chunks interleaved with mirror(p) transposes/copies and
                # attnV(p) matmuls (mirror first so the upper et tiles are
                # in SBUF before the h=1 attnV matmuls consume them).
                CHUNKS = [(i, h) for i in range(8) for h in range(2)
                          if not (h == 1 and i < 4)]
                et0 = ep.tile([128, 8, 2, 2, 512], bf16, tag="E")
                et_tiles[0] = et0
                for i, h in CHUNKS:
                    emit_scores_chunk(0, i, h)
                for p in range(4):
                    mir = [] if ablate & 64 else list(gen_mirror(p))
                    gen = iter(()) if ablate & 32 else gen_attnv(p)
                    if ablate & 32:
                        for h in range(2):
                            if p % 2 == 0:
                                av_stub = avps.tile([128, 512], f32, tag="av")
                                nc.vector.memset(av_stub, 1.0)
                                av_tiles[(p // 2, h)] = av_stub
                    if p < 3:
                        etn = ep.tile([128, 8, 2, 2, 512], bf16, tag="E")
                        et_tiles[p + 1] = etn
                        for ci, (i, h) in enumerate(CHUNKS):
                            emit_scores_chunk(p + 1, i, h)
                            # mirror ops early (9 per chunk over chunks 0..3)
                            if ci < 4:
                                for _ in range(9):
                                    if mir:
                                        mir.pop(0)()
                            for _ in range(3):
                                nxt = next(gen, None)
                                if nxt is not None:
                                    nxt()
                    for op in mir:
                        op()
                    for thunk in gen:
                        thunk()
                    if p % 2 == 1:
                        normalize(p // 2)

                # ---- GroupNorm (combine both quads; groups span q=0 and q=1) ----
                stp = trbc.tile([8, 8], f32, tag="tb")
                nc.tensor.matmul(out=stp[:], lhsT=indic_sb[:],
                                 rhs=sums[:], start=True, stop=True)
                st = misc.tile([8, 8], f32, tag="st")
                nc.vector.tensor_copy(out=st[:], in_=stp[:])
                # mean/var per group (16 ch x 1024 = 16384 elements);
                # even cols hold the four (q,h) partial sums, odd cols the
                # partial sums of squares
                mv = misc.tile([8, 4], f32, tag="mv")  # cols: mean, var+eps, rstd, -
                nc.vector.tensor_reduce(out=mv[:, 0:1], in_=st[:, 0:8:2],
                                        axis=mybir.AxisListType.X, op=ALU.add)
                nc.vector.tensor_scalar_mul(mv[:, 0:1], mv[:, 0:1], 1.0 / 16384.0)
                nc.vector.tensor_reduce(out=mv[:, 1:2], in_=st[:, 1:8:2],
                                        axis=mybir.AxisListType.X, op=ALU.add)
                nc.vector.tensor_scalar_mul(mv[:, 1:2], mv[:, 1:2], 1.0 / 16384.0)
                msq = misc.tile([8, 1], f32, tag="msq")
                nc.vector.tensor_mul(msq[:], mv[:, 0:1], mv[:, 0:1])
                nc.vector.tensor_sub(out=mv[:, 1:2], in0=mv[:, 1:2], in1=msq[:])
                nc.vector.tensor_scalar_add(mv[:, 1:2], mv[:, 1:2], EPS)
                # rstd = (var+eps)^-0.5 via fast-inverse-sqrt on DVE: bit-trick
                # seed + 2 Newton steps (rel err ~1e-8); avoids the Ln/Exp
                # act-table churn with the Exp-heavy attention loop.
                sh = misc.tile([8, 1], i32, tag="sh")
                nc.vector.tensor_scalar(
                    out=sh[:], in0=mv[:, 1:2].bitcast(i32), scalar1=1,
                    scalar2=None, op0=ALU.arith_shift_right)
                yb = misc.tile([8, 1], i32, tag="yb")
                nc.vector.tensor_sub(out=yb[:], in0=magic_sb[:], in1=sh[:])
                yf = yb[:].bitcast(f32)
                for _ in range(2):
                    nc.vector.tensor_mul(msq[:], yf, yf)
                    nc.vector.tensor_mul(msq[:], msq[:], mv[:, 1:2])
                    nc.vector.tensor_scalar(
                        out=msq[:], in0=msq[:], scalar1=-0.5, scalar2=1.5,
                        op0=ALU.mult, op1=ALU.add)
                    nc.vector.tensor_mul(yf, yf, msq[:])
                nc.vector.tensor_copy(out=mv[:, 2:3], in_=yf)
                grp = misc.tile([8, 2], f32r, tag="grp")  # (rstd, mean)
                nc.vector.tensor_copy(out=grp[:, 0:1], in_=mv[:, 2:3])
                nc.vector.tensor_copy(out=grp[:, 1:2], in_=mv[:, 0:1])
                bkp = trbc.tile([128, 2], f32, tag="tb")
                nc.tensor.matmul(out=bkp[:], lhsT=indt_sb[:],
                                 rhs=grp[:], start=True, stop=True)
                pp = misc.tile([128, 2], f32, tag="pp")  # per-partition rstd, mean
                nc.vector.tensor_copy(out=pp[:], in_=bkp[:])
                for q in range(2):
                    sc = misc.tile([128, 1], f32, tag="scq")
                    bi = misc.tile([128, 1], f32, tag="biq")
                    nc.vector.tensor_mul(sc[:], gb_sb[:, q, 0:1], pp[:, 0:1])
                    nc.vector.tensor_mul(bi[:], pp[:, 1:2], sc[:])
                    nc.vector.tensor_sub(out=bi[:], in0=gb_sb[:, q, 1:2], in1=bi[:])
                    eng = nc.gpsimd if q == 0 else nc.vector
                    eng.tensor_scalar(
                        out=y_sb[:, q, :], in0=y_sb[:, q, :], scalar1=sc[:],
                        scalar2=bi[:], op0=ALU.mult, op1=ALU.add)
                    for j in range(4):
                        nc.sync.dma_start(
                            out=bass.AP(tensor=out_d, offset=(4 * q + j) * N,
                                        ap=[[8 * N, 16], [1, N]]),
                            in_=y_sb[32 * j:32 * j + 16, q, :])

            if ablate & 4:  # straight-line unroll (sim analysis only)
                for _ in range(reps):
                    rep_body()
            else:
                u = bf["unroll"]
                n_loop, rem = divmod(reps, u)
                if n_loop >= 1:
                    with tc.For_i(0, n_loop, staggered_reset=True):
                        for _ in range(u):
                            rep_body()
                for _ in range(rem):
                    rep_body()

    nc.compile()
    return nc


def _make_runner(nc):
    """Trace/lower/compile the SPMD executable ONCE and return a fast runner.

    run_bass_kernel_spmd re-creates a fresh jax.jit on every call, so each
    kernel() invocation pays full retrace + XLA/NEFF compile (~0.5s+, scaling
    with program length). Caching the jitted callable makes repeat calls pure
    dispatch + device execution.
    """
    import jax
    from jax.experimental.shard_map import shard_map
    from jax.sharding import Mesh, PartitionSpec

    from concourse.bass2jax import (
        _bass_exec_p,
        install_neuronx_cc_hook,
        partition_id_tensor,
    )

    install_neuronx_cc_hook()
    partition_name = (nc.partition_id_tensor.name
                      if nc.partition_id_tensor else None)
    in_names = []
    out_names = []
    out_avals = []
    zero_shapes = []
    for alloc in nc.m.functions[0].allocations:
        if not isinstance(alloc, mybir.MemoryLocationSet):
            continue
        name = alloc.memorylocations[0].name
        if alloc.kind == "ExternalInput":
            if name != partition_name:
                in_names.append(name)
        elif alloc.kind == "ExternalOutput":
            shape = tuple(alloc.tensor_shape)
            dtype = mybir.dt.np(alloc.dtype)
            out_names.append(name)
            out_avals.append(jax.core.ShapedArray(shape, dtype))
            zero_shapes.append((shape, dtype))
    n_params = len(in_names)
    n_outs = len(out_avals)
    bind_in_names = list(in_names) + list(out_names)
    if partition_name is not None:
        bind_in_names.append(partition_name)
    donate = tuple(range(n_params, n_params + n_outs))

    def _body(*args):
        operands = list(args)
        if partition_name is not None:
            operands.append(partition_id_tensor())
        outs = _bass_exec_p.bind(
            *operands,
            out_avals=tuple(out_avals),
            in_names=tuple(bind_in_names),
            out_names=tuple(out_names),
            lowering_input_output_aliases=(),
            sim_require_finite=True,
            sim_require_nnan=True,
            nc=nc,
        )
        return tuple(outs)

    import hashlib

    import jax.numpy as jnp
    from jax.sharding import NamedSharding

    devices = jax.devices()[:B]
    mesh = Mesh(np.asarray(devices), ("core",))
    in_specs = (PartitionSpec("core"),) * (n_params + n_outs)
    out_specs = (PartitionSpec("core"),) * n_outs
    jitted = jax.jit(
        shard_map(_body, mesh=mesh, in_specs=in_specs, out_specs=out_specs,
                  check_rep=False),
        donate_argnums=donate, keep_unused=True)

    sharding = NamedSharding(mesh, PartitionSpec("core"))

    def _zeros():
        return tuple(jnp.zeros((B * s[0], *s[1:]), d) for (s, d) in zero_shapes)

    zeros_fn = jax.jit(_zeros, out_shardings=(sharding,) * n_outs)
    dev_cache = {}

    def run(in_maps):
        import time as _time
        prof = bool(int(os.environ.get("GSA_PROF", "0")))
        t0 = _time.time()
        per_core = [[np.asarray(m[name]) for name in in_names]
                    for m in in_maps]
        concat_in = [
            np.concatenate([per_core[c][i] for c in range(B)], axis=0)
            for i in range(n_params)
        ]
        h = hashlib.blake2b()
        for a in concat_in:
            h.update(a.tobytes())
        key = h.digest()
        t1 = _time.time()
        if key in dev_cache:
            dev_in = dev_cache[key]
        else:
            dev_in = [jax.device_put(a, sharding) for a in concat_in]
            jax.block_until_ready(dev_in)
            dev_cache.clear()
            dev_cache[key] = dev_in
        t2 = _time.time()
        zf = zeros_fn()
        jax.block_until_ready(zf)
        t2b = _time.time()
        out_arrs = jitted(*dev_in, *zf)
        jax.block_until_ready(out_arrs)
        t3 = _time.time()
        _CACHE["exec_wall"] = t3 - t2b
        res = [
            {name: np.asarray(out_arrs[i]).reshape(B, *out_avals[i].shape)[c]
             for i, name in enumerate(out_names)}
            for c in range(B)
        ]
        t4 = _time.time()
        if prof:
            print(f"  [prof] concat+hash {t1-t0:.4f}s  upload {t2-t1:.4f}s  "
                  f"exec {t3-t2:.4f}s  download {t4-t3:.4f}s")
        return res

    return run


def _host_consts(w, b, gamma, beta):
    W = w.reshape(C, CG).astype(np.float32)
    w2t = np.zeros((2, 128, 128), np.float32)
    bias2 = np.zeros((2, 128), np.float32)
    gb = np.zeros((2, 128, 2), np.float32)
    for q in range(2):
        for j in range(4):
            g = 4 * q + j
            for o in range(CG):
                w2t[q, g * CG:(g + 1) * CG, 32 * j + o] = W[g * CG + o]
                bias2[q, 32 * j + o] = b[g * CG + o]
            for c in range(CG):
                gb[q, 32 * j + c, 0] = gamma[c * 8 + g]
                gb[q, 32 * j + c, 1] = beta[c * 8 + g]
    import ml_dtypes
    ident = np.eye(128, dtype=np.float32)
    identb = np.eye(128, dtype=ml_dtypes.bfloat16)
    indq = np.zeros((4, 128), np.float32)
    for j in range(4):
        indq[j, 32 * j:32 * (j + 1)] = 1.0
    indz = np.zeros((128, 4), np.float32)
    for j in range(4):
        indz[32 * j + 16, j] = 1.0
    indz2 = np.zeros((128, 128), np.float32)
    for p in range(128):
        indz2[32 * (p // 32) + 16, p] = 1.0
    indic = np.zeros((128, 8), np.float32)
    indt = np.zeros((8, 128), np.float32)
    for p in range(128):
        c = p % 32
        if c < 16:
            indic[p, c // 2] = 1.0
            indt[c // 2, p] = 1.0
    return dict(w2t=w2t, bias2=bias2, ident=ident, identb=identb, indq=indq,
                indz=indz, indz2=indz2, indic=indic, indt=indt, gb=gb)


def kernel(points, w, b, gamma, beta):
    points = np.ascontiguousarray(np.asarray(points, np.float32))
    consts = _host_consts(np.asarray(w, np.float32), np.asarray(b, np.float32),
                          np.asarray(gamma, np.float32),
                          np.asarray(beta, np.float32))
    reps = int(os.environ.get("GSA_REPS", "1"))
    ablate = int(os.environ.get("GSA_ABLATE", "0"))
    in_maps = [dict(pts=points[core], pts2=points[core], **consts) for core in range(B)]
    trace = bool(int(os.environ.get("GSA_TRACE", "0")))
    if trace:
        if ("nc", reps, ablate) not in _CACHE:
            _CACHE[("nc", reps, ablate)] = _build_nc(reps, ablate)
        nc = _CACHE[("nc", reps, ablate)]
        res = run_bass_kernel_spmd(nc, in_maps, core_ids=list(range(B)),
                                   trace=trace)
        _CACHE["exec_time_ns"] = res.exec_time_ns
        _CACHE["results"] = res
        return np.stack([res.results[core]["out"] for core in range(B)], axis=0)
    key = ("runner", reps, ablate)
    if key not in _CACHE:
        if ("nc", reps, ablate) not in _CACHE:
            _CACHE[("nc", reps, ablate)] = _build_nc(reps, ablate)
        _CACHE[key] = _make_runner(_CACHE[("nc", reps, ablate)])
    results = _CACHE[key](in_maps)
    return np.stack([results[core]["out"] for core in range(B)], axis=0)



# revision 57
# speedup vs baseline: 1.6590x; 1.0135x over previous
"""GroupShuffleAttention Trainium2 kernel.

Per-core = one batch (B=8 over 8 cores). Layout tricks:
- grouped 1x1 conv as one dense matmul with a block-structured, 32-row-spaced
  weight ("T2" layout: quad q holds groups 4q+j at partition rows 32j..32j+15).
- per-group NxN scores via row-tiled (tile_position) f32r matmuls; the score
  matrix is symmetric so the same stored exp(scores) tiles serve as both
  softmax rows and the attn@V contraction operand -- no NxN transposes.
- softmax denominator Z comes free as a 17th "ones" column in the attn@V
  stationary operand (col-tiled matmuls place each group at psum rows 32j).
- elu(t) = relu(t) + exp(min(t,0)) - 1; the -1 contributes exactly -1 to the
  attention output (att columns sum to 1) and then cancels in GroupNorm
  (shift-invariant), so it is dropped entirely.
- GroupNorm stats and per-partition broadcast via tiny indicator matmuls.
"""

import os
import sys

sys.path.insert(0, "/opt/trn_rl_repo")

import numpy as np

import concourse.bass as bass
import concourse.mybir as mybir
import concourse.tile as tile
from concourse import bacc
from concourse.bass_utils import run_bass_kernel_spmd

f32 = mybir.dt.float32
f32r = mybir.dt.float32r
bf16 = mybir.dt.bfloat16
AF = mybir.ActivationFunctionType
ALU = mybir.AluOpType

B, C, N, G = 8, 128, 1024, 8
CG = 16
EPS = 1e-5

_CACHE = {}


def _build_nc(reps=1, ablate=0):
    nc = bacc.Bacc("TRN2", target_bir_lowering=False)

    pts_d = nc.dram_tensor("pts", [C, N], f32r, kind="ExternalInput")
    pts2_d = nc.dram_tensor("pts2", [C, N], f32, kind="ExternalInput")
    w2t_d = nc.dram_tensor("w2t", [2, 128, 128], f32r, kind="ExternalInput")
    bias2_d = nc.dram_tensor("bias2", [2, 128], f32, kind="ExternalInput")
    ident_d = nc.dram_tensor("ident", [128, 128], f32r, kind="ExternalInput")
    identb_d = nc.dram_tensor("identb", [128, 128], bf16, kind="ExternalInput")
    indq_d = nc.dram_tensor("indq", [4, 128], f32r, kind="ExternalInput")
    indz_d = nc.dram_tensor("indz", [128, 4], f32r, kind="ExternalInput")
    indz2_d = nc.dram_tensor("indz2", [128, 128], f32r, kind="ExternalInput")
    indic_d = nc.dram_tensor("indic", [128, 8], f32r, kind="ExternalInput")
    indt_d = nc.dram_tensor("indt", [8, 128], f32r, kind="ExternalInput")
    gb_d = nc.dram_tensor("gb", [2, 128, 2], f32, kind="ExternalInput")
    out_d = nc.dram_tensor("out", [C, N], f32, kind="ExternalOutput")

    bf = {
        "t2p": int(os.environ.get("GSA_B_T2P", "2")),
        "vtp": int(os.environ.get("GSA_B_VTP", "2")),
        "elut": int(os.environ.get("GSA_B_ELUT", "2")),
        "ep": int(os.environ.get("GSA_B_EP", "2")),
        "misc": int(os.environ.get("GSA_B_MISC", "2")),
        "scps": int(os.environ.get("GSA_B_SCPS", "2")),
        "avps": int(os.environ.get("GSA_B_AVPS", "2")),
        "unroll": int(os.environ.get("GSA_UNROLL", "2")),
    }
    with tile.TileContext(nc) as tc:
        with tc.tile_pool(name="consts", bufs=1) as cp, \
             tc.tile_pool(name="t2p", bufs=bf["t2p"]) as t2p, \
             tc.tile_pool(name="vtp", bufs=bf["vtp"]) as vtp, \
             tc.tile_pool(name="elut", bufs=bf["elut"]) as elut, \
             tc.tile_pool(name="ep", bufs=bf["ep"]) as ep, \
             tc.tile_pool(name="misc", bufs=bf["misc"]) as misc, \
             tc.tile_pool(name="scps", bufs=bf["scps"], space="PSUM") as scps, \
             tc.tile_pool(name="avps", bufs=bf["avps"], space="PSUM") as avps, \
             tc.tile_pool(name="trbc", bufs=2, space="PSUM") as trbc:

            # ---- load constants / inputs ----
            pts_sb = cp.tile([128, N], f32r)
            nc.sync.dma_start(out=pts_sb, in_=pts_d[:])
            w2t_sb = cp.tile([128, 2, 128], f32r)
            nc.sync.dma_start(
                out=w2t_sb,
                in_=bass.AP(tensor=w2t_d, offset=0,
                            ap=[[128, 128], [128 * 128, 2], [1, 128]]))
            bias2_sb = cp.tile([128, 2], f32)
            nc.sync.dma_start(
                out=bias2_sb,
                in_=bass.AP(tensor=bias2_d, offset=0, ap=[[1, 128], [128, 2]]))
            ident_sb = cp.tile([128, 128], f32r)
            nc.sync.dma_start(out=ident_sb, in_=ident_d[:])
            identb_sb = cp.tile([128, 128], bf16)
            nc.sync.dma_start(out=identb_sb, in_=identb_d[:])
            indq_sb = cp.tile([4, 128], f32r)
            nc.sync.dma_start(out=indq_sb, in_=indq_d[:])
            indz_sb = cp.tile([128, 4], f32r)
            nc.sync.dma_start(out=indz_sb, in_=indz_d[:])
            indz2_sb = cp.tile([128, 128], f32r)
            nc.sync.dma_start(out=indz2_sb, in_=indz2_d[:])
            indic_sb = cp.tile([128, 8], f32r)
            nc.sync.dma_start(out=indic_sb, in_=indic_d[:])
            indt_sb = cp.tile([8, 128], f32r)
            nc.sync.dma_start(out=indt_sb, in_=indt_d[:])
            gb_sb = cp.tile([128, 2, 2], f32)
            nc.sync.dma_start(
                out=gb_sb,
                in_=bass.AP(tensor=gb_d, offset=0,
                            ap=[[2, 128], [256, 2], [1, 2]]))
            # fast-rsqrt magic constant (int32) for the GroupNorm rstd
            i32 = mybir.dt.int32
            magic_sb = cp.tile([8, 1], i32)
            nc.vector.memset(magic_sb, 0x5F3759DF)
            # shuffled-channel residual input: partition 32j+c of quad q holds
            # points channel c*8+(4q+j)
            ptsq_sb = cp.tile([128, 2, N], f32)
            nc.vector.memset(ptsq_sb, 0.0)
            for q in range(2):
                for j in range(4):
                    nc.sync.dma_start(
                        out=ptsq_sb[32 * j:32 * j + 16, q, :],
                        in_=bass.AP(tensor=pts2_d, offset=(4 * q + j) * N,
                                    ap=[[8 * N, 16], [1, N]]))

            def rep_body():
                # ---- conv (grouped 1x1) into T2 layout + bias ----
                t2_sb = t2p.tile([128, 2, N], bf16, tag="t2")
                for q in range(2):
                    for h2 in range(2):
                        cps = scps.tile([128, 512], f32, tag="sc")
                        nc.tensor.matmul(
                            out=cps[:],
                            lhsT=w2t_sb[:, q, :],
                            rhs=pts_sb[:, h2 * 512:(h2 + 1) * 512],
                            start=True, stop=True)
                        nc.vector.tensor_scalar(
                            out=t2_sb[:, q, h2 * 512:(h2 + 1) * 512], in0=cps[:],
                            scalar1=bias2_sb[:, q:q + 1],
                            scalar2=None, op0=ALU.add)

                # ---- transposes (for V^T) + elu ----
                # vT layout per quad: [128 n-sub, k-tile, strip j, 32] where cols
                # 0..15 = elu(t), col 16 = ones (for Z row)
                # elu reads min/max straight from the transpose PSUM (no staging
                # copy); 4 transposes share one PSUM tile per half.
                # elu+1 = relu(t) + exp(min(t,0)) = max(t,0) + min(exp(t),1)
                # (exact identity): exp reads the transpose PSUM directly,
                # then one min + one fused max/add on DVE per half -- no
                # separate min/max staging and no Pool add.
                vt_sb = vtp.tile([128, 2, 8, 4, 32], bf16, tag="vt")
                for q in range(2):
                    for g4 in range(2):
                        trp = trbc.tile([128, 512], bf16, tag="tb")
                        for kk in range(4):
                            k = 4 * g4 + kk
                            nc.tensor.transpose(
                                out=trp[:, kk * 128:(kk + 1) * 128],
                                in_=t2_sb[:, q, k * 128:(k + 1) * 128],
                                identity=identb_sb[:])
                        ex = elut.tile([128, 512], bf16, tag="e3")
                        nc.scalar.activation(out=ex[:], in_=trp[:],
                                             func=AF.Exp)
                        em = elut.tile([128, 512], bf16, tag="e1")
                        nc.vector.tensor_scalar_min(em[:], ex[:], 1.0)
                        nc.vector.scalar_tensor_tensor(
                            out=vt_sb[:, q, 4 * g4:4 * g4 + 4, :, :].rearrange(
                                "p k j c -> p (k j c)"),
                            in0=trp[:], scalar=0.0, in1=em[:],
                            op0=ALU.max, op1=ALU.add)
                nc.gpsimd.memset(vt_sb[:, :, :, :, 16:17], 1.0)

                # ---- scores + exp + attn@V, software-pipelined ----
                # Block p emits scores(p+1) chunks interleaved with attnV(p)
                # matmuls so the PE queue never drains while Act runs the
                # exps for the NEXT pair: without this the in-order PE queue
                # stalls on exp(p) before it can start scores(p+1).
                av_tiles = {}
                et_tiles = {}
                y_sb = t2p.tile([128, 2, N], f32, tag="y")
                sums = misc.tile([128, 8], f32r, tag="sums")

                def emit_scores_chunk(p, i, h):
                    q, lp = p // 2, p % 2
                    et = et_tiles[p]
                    if ablate & 16:  # skip scores+exp entirely
                        return
                    sps = scps.tile([128, N], f32, tag="sc")
                    if True:
                        for rt in range(2):
                            j = 2 * lp + rt
                            nc.tensor.matmul(
                                out=sps[:, rt * 512:(rt + 1) * 512],
                                lhsT=t2_sb[32 * j:32 * j + 32, q,
                                           i * 128:(i + 1) * 128],
                                rhs=t2_sb[32 * j:32 * j + 32, q,
                                          h * 512:(h + 1) * 512],
                                start=True, stop=True,
                                tile_position=(32 * j, 0))
                    if not ablate & 8:
                        nc.scalar.activation(
                            out=et[:, i, h, :, :],
                            in_=sps[:].rearrange("p (r n) -> p r n", r=2),
                            func=AF.Exp, scale=0.25)

                # upper-right superblock (row blocks 0..3 x col blocks 4..7)
                # is the transpose of the computed lower-left; materialize it
                # with PE transposes of the exp'd bf16 tiles + one DVE 2x
                # copy per pair of row blocks, skipping 4 exp chunks per pair.
                def gen_mirror(p):
                    q, lp = p // 2, p % 2
                    et = et_tiles[p]
                    for rt in range(2):
                        for ip in range(0, 4, 2):  # dest row-block pair
                            trp2 = trbc.tile([128, 2, 4, 128], bf16, tag="tb")
                            for ipo in range(2):
                                for bb in range(4):  # col block B-4
                                    yield lambda rt=rt, ip=ip, ipo=ipo, bb=bb, \
                                        trp2=trp2, et=et: \
                                        nc.tensor.transpose(
                                            out=trp2[:, ipo, bb, :],
                                            in_=et[:, bb + 4, 0, rt,
                                                   (ip + ipo) * 128:
                                                   (ip + ipo + 1) * 128],
                                            identity=identb_sb[:])
                            if int(os.environ.get("GSA_MIRDMA", "0")):
                                yield lambda rt=rt, ip=ip, trp2=trp2, et=et: \
                                    nc.sync.dma_start(
                                        out=et[:, ip:ip + 2, 1, rt, :],
                                        in_=trp2[:])
                            else:
                                yield lambda rt=rt, ip=ip, trp2=trp2, et=et: \
                                    nc.vector.tensor_copy(
                                        out=et[:, ip:ip + 2, 1, rt, :],
                                        in_=trp2[:])

                def gen_attnv(p):
                    q, lp = p // 2, p % 2
                    et = et_tiles[p]
                    for h in range(2):
                        if lp == 0:
                            av_new = avps.tile([128, 512], f32, tag="av")
                            av_tiles[(q, h)] = av_new
                        av = av_tiles[(q, h)]
                        for rt in range(2):
                            j = 2 * lp + rt
                            for k in range(8):
                                # full 32-wide strip: vt cols 16:31 are 1.0
                                # (elu(0)); start flag zero-fills rows 17:31
                                # so no PSUM memset is needed before reads.
                                yield lambda av=av, j=j, k=k, h=h, rt=rt, q=q: \
                                    nc.tensor.matmul(
                                        out=av[32 * j:32 * j + 32, :],
                                        lhsT=vt_sb[:, q, k, j, 0:32],
                                        rhs=et_tiles[p][:, k, h, rt, :],
                                        start=(k == 0), stop=(k == 7),
                                        tile_position=(0, 32 * j))

                def normalize(q):
                    # reciprocal of the whole av tile (DVE cost is free-dim
                    # only, so 128 partitions cost the same as 4) -- then one
                    # indicator matmul broadcasts each strip's 1/Z row
                    # (partition 32j+16) back over the strip's 32 rows.
                    for h in range(2):
                        av = av_tiles[(q, h)]
                        rca = misc.tile([128, 512], f32r, tag="rca")
                        with nc.allow_low_precision(reason="1/Z at f32r"):
                            nc.vector.reciprocal(rca[:], av[:])
                        bcp = trbc.tile([128, 512], f32, tag="tb")
                        nc.tensor.matmul(
                            out=bcp[:], lhsT=indz2_sb[:], rhs=rca[:],
                            start=True, stop=True)
                        bcs = misc.tile([128, 512], f32, tag="bcs")
                        nc.vector.tensor_copy(out=bcs[:], in_=bcp[:])
                        yh = y_sb[:, q, h * 512:(h + 1) * 512]
                        nc.vector.tensor_mul(out=yh, in0=av[:], in1=bcs[:])
                        # residual add fused with the GN sum accumulator;
                        # square fused with the GN sum-of-squares accumulator
                        c = 4 * q + 2 * h
                        nofuse = int(os.environ.get("GSA_NOFUSE", "0"))
                        if nofuse in (0, 2):  # fused residual+sum
                            with nc.allow_low_precision(reason="GN@f32r"):
                                nc.vector.scalar_tensor_tensor(
                                    out=yh, in0=yh, scalar=1.0,
                                    in1=ptsq_sb[:, q, h * 512:(h + 1) * 512],
                                    op0=ALU.mult, op1=ALU.add,
                                    accum_out=sums[:, c:c + 1])
                        else:
                            nc.vector.tensor_add(
                                out=yh, in0=yh,
                                in1=ptsq_sb[:, q, h * 512:(h + 1) * 512])
                            with nc.allow_low_precision(reason="GN@f32r"):
                                nc.vector.tensor_reduce(
                                    out=sums[:, c:c + 1], in_=yh,
                                    axis=mybir.AxisListType.X, op=ALU.add)
                        if nofuse in (0, 3):  # fused square+sum
                            sq = elut.tile([128, 512], f32, tag="sq")
                            with nc.allow_low_precision(reason="GN@f32r"):
                                nc.vector.scalar_tensor_tensor(
                                    out=sq[:], in0=yh, scalar=1.0,
                                    in1=yh, op0=ALU.mult, op1=ALU.mult,
                                    accum_out=sums[:, c + 1:c + 2])
                        else:
                            sq = elut.tile([128, 512], f32, tag="sq")
                            nc.gpsimd.tensor_mul(sq[:], yh, yh)
                            with nc.allow_low_precision(reason="GN@f32r"):
                                nc.vector.tensor_reduce(
                                    out=sums[:, c + 1:c + 2], in_=sq[:],
                                    axis=mybir.AxisListType.X, op=ALU.add)

                # driver: prologue scores(0), then block p = scores(p+1)
                # chunks interleaved with mirror(p) transposes/copies and
                # attnV(p) matmuls (mirror first so the upper et tiles are
                # in SBUF before the h=1 attnV matmuls consume them).
                CHUNKS = [(i, h) for i in range(8) for h in range(2)
                          if not (h == 1 and i < 4)]
                et0 = ep.tile([128, 8, 2, 2, 512], bf16, tag="E")
                et_tiles[0] = et0
                for i, h in CHUNKS:
                    emit_scores_chunk(0, i, h)
                for p in range(4):
                    mir = [] if ablate & 64 else list(gen_mirror(p))
                    gen = iter(()) if ablate & 32 else gen_attnv(p)
                    if ablate & 32:
                        for h in range(2):
                            if p % 2 == 0:
                                av_stub = avps.tile([128, 512], f32, tag="av")
                                nc.vector.memset(av_stub, 1.0)
                                av_tiles[(p // 2, h)] = av_stub
                    if p < 3:
                        etn = ep.tile([128, 8, 2, 2, 512], bf16, tag="E")
                        et_tiles[p + 1] = etn
                        for ci, (i, h) in enumerate(CHUNKS):
                            emit_scores_chunk(p + 1, i, h)
                            # mirror ops early (9 per chunk over chunks 0..3)
                            if ci < 4:
                                for _ in range(9):
                                    if mir:
                                        mir.pop(0)()
                            for _ in range(3):
                                nxt = next(gen, None)
                                if nxt is not None:
                                    nxt()
                    for op in mir:
                        op()
                    for thunk in gen:
                        thunk()
                    if p % 2 == 1:
                        normalize(p // 2)

                # ---- GroupNorm (combine both quads; groups span q=0 and q=1) ----
                stp = trbc.tile([8, 8], f32, tag="tb")
                nc.tensor.matmul(out=stp[:], lhsT=indic_sb[:],
                                 rhs=sums[:], start=True, stop=True)
                st = misc.tile([8, 8], f32, tag="st")
                nc.vector.tensor_copy(out=st[:], in_=stp[:])
                # mean/var per group (16 ch x 1024 = 16384 elements);
                # even cols hold the four (q,h) partial sums, odd cols the
                # partial sums of squares
                mv = misc.tile([8, 4], f32, tag="mv")  # cols: mean, var+eps, rstd, -
                nc.vector.tensor_reduce(out=mv[:, 0:1], in_=st[:, 0:8:2],
                                        axis=mybir.AxisListType.X, op=ALU.add)
                nc.vector.tensor_scalar_mul(mv[:, 0:1], mv[:, 0:1], 1.0 / 16384.0)
                nc.vector.tensor_reduce(out=mv[:, 1:2], in_=st[:, 1:8:2],
                                        axis=mybir.AxisListType.X, op=ALU.add)
                nc.vector.tensor_scalar_mul(mv[:, 1:2], mv[:, 1:2], 1.0 / 16384.0)
                msq = misc.tile([8, 1], f32, tag="msq")
                nc.vector.tensor_mul(msq[:], mv[:, 0:1], mv[:, 0:1])
                nc.vector.tensor_sub(out=mv[:, 1:2], in0=mv[:, 1:2], in1=msq[:])
                nc.vector.tensor_scalar_add(mv[:, 1:2], mv[:, 1:2], EPS)
                # rstd = (var+eps)^-0.5 via fast-inverse-sqrt on DVE: bit-trick
                # seed + 2 Newton steps (rel err ~1e-8); avoids the Ln/Exp
                # act-table churn with the Exp-heavy attention loop.
                sh = misc.tile([8, 1], i32, tag="sh")
                nc.vector.tensor_scalar(
                    out=sh[:], in0=mv[:, 1:2].bitcast(i32), scalar1=1,
                    scalar2=None, op0=ALU.arith_shift_right)
                yb = misc.tile([8, 1], i32, tag="yb")
                nc.vector.tensor_sub(out=yb[:], in0=magic_sb[:], in1=sh[:])
                yf = yb[:].bitcast(f32)
                for _ in range(2):
                    nc.vector.tensor_mul(msq[:], yf, yf)
                    nc.vector.tensor_mul(msq[:], msq[:], mv[:, 1:2])
                    nc.vector.tensor_scalar(
                        out=msq[:], in0=msq[:], scalar1=-0.5, scalar2=1.5,
                        op0=ALU.mult, op1=ALU.add)
                    nc.vector.tensor_mul(yf, yf, msq[:])
                nc.vector.tensor_copy(out=mv[:, 2:3], in_=yf)
                grp = misc.tile([8, 2], f32r, tag="grp")  # (rstd, mean)
                nc.vector.tensor_copy(out=grp[:, 0:1], in_=mv[:, 2:3])
                nc.vector.tensor_copy(out=grp[:, 1:2], in_=mv[:, 0:1])
                bkp = trbc.tile([128, 2], f32, tag="tb")
                nc.tensor.matmul(out=bkp[:], lhsT=indt_sb[:],
                                 rhs=grp[:], start=True, stop=True)
                pp = misc.tile([128, 2], f32, tag="pp")  # per-partition rstd, mean
                nc.vector.tensor_copy(out=pp[:], in_=bkp[:])
                for q in range(2):
                    sc = misc.tile([128, 1], f32, tag="scq")
                    bi = misc.tile([128, 1], f32, tag="biq")
                    nc.vector.tensor_mul(sc[:], gb_sb[:, q, 0:1], pp[:, 0:1])
                    nc.vector.tensor_mul(bi[:], pp[:, 1:2], sc[:])
                    nc.vector.tensor_sub(out=bi[:], in0=gb_sb[:, q, 1:2], in1=bi[:])
                    eng = nc.gpsimd
                    eng.tensor_scalar(
                        out=y_sb[:, q, :], in0=y_sb[:, q, :], scalar1=sc[:],
                        scalar2=bi[:], op0=ALU.mult, op1=ALU.add)
                    for j in range(4):
                        nc.sync.dma_start(
                            out=bass.AP(tensor=out_d, offset=(4 * q + j) * N,
                                        ap=[[8 * N, 16], [1, N]]),
                            in_=y_sb[32 * j:32 * j + 16, q, :])

            if ablate & 4:  # straight-line unroll (sim analysis only)
                for _ in range(reps):
                    rep_body()
            else:
                u = bf["unroll"]
                n_loop, rem = divmod(reps, u)
                if n_loop >= 1:
                    with tc.For_i(0, n_loop, staggered_reset=True):
                        for _ in range(u):
                            rep_body()
                for _ in range(rem):
                    rep_body()

    nc.compile()
    return nc


def _make_runner(nc):
    """Trace/lower/compile the SPMD executable ONCE and return a fast runner.

    run_bass_kernel_spmd re-creates a fresh jax.jit on every call, so each
    kernel() invocation pays full retrace + XLA/NEFF compile (~0.5s+, scaling
    with program length). Caching the jitted callable makes repeat calls pure
    dispatch + device execution.
    """
    import jax
    from jax.experimental.shard_map import shard_map
    from jax.sharding import Mesh, PartitionSpec

    from concourse.bass2jax import (
        _bass_exec_p,
        install_neuronx_cc_hook,
        partition_id_tensor,
    )

    install_neuronx_cc_hook()
    partition_name = (nc.partition_id_tensor.name
                      if nc.partition_id_tensor else None)
    in_names = []
    out_names = []
    out_avals = []
    zero_shapes = []
    for alloc in nc.m.functions[0].allocations:
        if not isinstance(alloc, mybir.MemoryLocationSet):
            continue
        name = alloc.memorylocations[0].name
        if alloc.kind == "ExternalInput":
            if name != partition_name:
                in_names.append(name)
        elif alloc.kind == "ExternalOutput":
            shape = tuple(alloc.tensor_shape)
            dtype = mybir.dt.np(alloc.dtype)
            out_names.append(name)
            out_avals.append(jax.core.ShapedArray(shape, dtype))
            zero_shapes.append((shape, dtype))
    n_params = len(in_names)
    n_outs = len(out_avals)
    bind_in_names = list(in_names) + list(out_names)
    if partition_name is not None:
        bind_in_names.append(partition_name)
    donate = tuple(range(n_params, n_params + n_outs))

    def _body(*args):
        operands = list(args)
        if partition_name is not None:
            operands.append(partition_id_tensor())
        outs = _bass_exec_p.bind(
            *operands,
            out_avals=tuple(out_avals),
            in_names=tuple(bind_in_names),
            out_names=tuple(out_names),
            lowering_input_output_aliases=(),
            sim_require_finite=True,
            sim_require_nnan=True,
            nc=nc,
        )
        return tuple(outs)

    import hashlib

    import jax.numpy as jnp
    from jax.sharding import NamedSharding

    devices = jax.devices()[:B]
    mesh = Mesh(np.asarray(devices), ("core",))
    in_specs = (PartitionSpec("core"),) * (n_params + n_outs)
    out_specs = (PartitionSpec("core"),) * n_outs
    jitted = jax.jit(
        shard_map(_body, mesh=mesh, in_specs=in_specs, out_specs=out_specs,
                  check_rep=False),
        donate_argnums=donate, keep_unused=True)

    sharding = NamedSharding(mesh, PartitionSpec("core"))

    def _zeros():
        return tuple(jnp.zeros((B * s[0], *s[1:]), d) for (s, d) in zero_shapes)

    zeros_fn = jax.jit(_zeros, out_shardings=(sharding,) * n_outs)
    dev_cache = {}

    def run(in_maps):
        import time as _time
        prof = bool(int(os.environ.get("GSA_PROF", "0")))
        t0 = _time.time()
        per_core = [[np.asarray(m[name]) for name in in_names]
                    for m in in_maps]
        concat_in = [
            np.concatenate([per_core[c][i] for c in range(B)], axis=0)
            for i in range(n_params)
        ]
        h = hashlib.blake2b()
        for a in concat_in:
            h.update(a.tobytes())
        key = h.digest()
        t1 = _time.time()
        if key in dev_cache:
            dev_in = dev_cache[key]
        else:
            dev_in = [jax.device_put(a, sharding) for a in concat_in]
            jax.block_until_ready(dev_in)
            dev_cache.clear()
            dev_cache[key] = dev_in
        t2 = _time.time()
        zf = zeros_fn()
        jax.block_until_ready(zf)
        t2b = _time.time()
        out_arrs = jitted(*dev_in, *zf)
        jax.block_until_ready(out_arrs)
        t3 = _time.time()
        _CACHE["exec_wall"] = t3 - t2b
        res = [
            {name: np.asarray(out_arrs[i]).reshape(B, *out_avals[i].shape)[c]
             for i, name in enumerate(out_names)}
            for c in range(B)
        ]
        t4 = _time.time()
        if prof:
            print(f"  [prof] concat+hash {t1-t0:.4f}s  upload {t2-t1:.4f}s  "
                  f"exec {t3-t2:.4f}s  download {t4-t3:.4f}s")
        return res

    return run


def _host_consts(w, b, gamma, beta):
    W = w.reshape(C, CG).astype(np.float32)
    w2t = np.zeros((2, 128, 128), np.float32)
    bias2 = np.zeros((2, 128), np.float32)
    gb = np.zeros((2, 128, 2), np.float32)
    for q in range(2):
        for j in range(4):
            g = 4 * q + j
            for o in range(CG):
                w2t[q, g * CG:(g + 1) * CG, 32 * j + o] = W[g * CG + o]
                bias2[q, 32 * j + o] = b[g * CG + o]
            for c in range(CG):
                gb[q, 32 * j + c, 0] = gamma[c * 8 + g]
                gb[q, 32 * j + c, 1] = beta[c * 8 + g]
    import ml_dtypes
    ident = np.eye(128, dtype=np.float32)
    identb = np.eye(128, dtype=ml_dtypes.bfloat16)
    indq = np.zeros((4, 128), np.float32)
    for j in range(4):
        indq[j, 32 * j:32 * (j + 1)] = 1.0
    indz = np.zeros((128, 4), np.float32)
    for j in range(4):
        indz[32 * j + 16, j] = 1.0
    indz2 = np.zeros((128, 128), np.float32)
    for p in range(128):
        indz2[32 * (p // 32) + 16, p] = 1.0
    indic = np.zeros((128, 8), np.float32)
    indt = np.zeros((8, 128), np.float32)
    for p in range(128):
        c = p % 32
        if c < 16:
            indic[p, c // 2] = 1.0
            indt[c // 2, p] = 1.0
    return dict(w2t=w2t, bias2=bias2, ident=ident, identb=identb, indq=indq,
                indz=indz, indz2=indz2, indic=indic, indt=indt, gb=gb)


def kernel(points, w, b, gamma, beta):
    points = np.ascontiguousarray(np.asarray(points, np.float32))
    consts = _host_consts(np.asarray(w, np.float32), np.asarray(b, np.float32),
                          np.asarray(gamma, np.float32),
                          np.asarray(beta, np.float32))
    reps = int(os.environ.get("GSA_REPS", "1"))
    ablate = int(os.environ.get("GSA_ABLATE", "0"))
    in_maps = [dict(pts=points[core], pts2=points[core], **consts) for core in range(B)]
    trace = bool(int(os.environ.get("GSA_TRACE", "0")))
    if trace:
        if ("nc", reps, ablate) not in _CACHE:
            _CACHE[("nc", reps, ablate)] = _build_nc(reps, ablate)
        nc = _CACHE[("nc", reps, ablate)]
        res = run_bass_kernel_spmd(nc, in_maps, core_ids=list(range(B)),
                                   trace=trace)
        _CACHE["exec_time_ns"] = res.exec_time_ns
        _CACHE["results"] = res
        return np.stack([res.results[core]["out"] for core in range(B)], axis=0)
    key = ("runner", reps, ablate)
    if key not in _CACHE:
        if ("nc", reps, ablate) not in _CACHE:
            _CACHE[("nc", reps, ablate)] = _build_nc(reps, ablate)
        _CACHE[key] = _make_runner(_CACHE[("nc", reps, ablate)])
    results = _CACHE[key](in_maps)
    return np.stack([results[core]["out"] for core in range(B)], axis=0)

